# revision 1
# baseline (speedup 1.0000x reference)
"""Trainium2 Bass kernel for ExampleGuidedAttention (N=8, C=256, H=W=64).

Data-parallel over batch N across 8 NeuronCores; each core computes one
batch element's full guided attention.

Algorithm notes (per core):
  q = conv_w @ src_pix                      [64, 4096]   (PE, bf16)
  S^T[j,i] = sum_o q[o,j] q[o,i]            (PE, bf16; S symmetric; two
             j-blocks packed in the 128x128 array via tile_position
             (row groups 0-63 / 64-127) since the contraction is only 64)
  F[j,i] = exp(S^T[j,i] - 64)               (ACT; global shift keeps fp32
             exp in range -- softmax ratio unchanged; diag scores are
             chi2(64) so they reach ~120).  The ACT accumulator
             (accum_out) yields Z partials for free: Z[j] = sum_i F[j,i]
             equals the softmax denominator because S is symmetric.
  O[c,i] = sum_j pixT[j,c] * F[j,i]         (PE, bf16, natural layout)
  out    = [ (1-m)*ref_att*invZ + m*ref ; src_att*invZ ]

Performance structure (vs the v1 kernel):
  - inputs land as 8KB-contiguous partition lines ([128, 2048] convert
    chunks): ~2x the bandwidth of the 2KB-line version.  NOTE: 16KB
    lines silently corrupt (descriptor length-field limit) -- keep
    convert-DMA lines at <= 8KB.
  - all [128, HW] column-broadcasts (mask, 1/Z) are ones-vector matmuls
    on the PE into PSUM -- the partition_broadcast DMA path is
    software-dynamic and costs ~25us per 2MB.
  - Z comes free from the exp pass via the ACT accumulator.
  - finalize is restructured as  flow = [(1-m)*ra]*izb + m*ref  with
    (1-m) folded into the PSUM->SBUF copy-out of each slice and m*ref
    precomputed during the input phase, so the tail after the last
    apply matmul is only slice 7's small DVE ops + 4 small DMAs.
  - scores for slice s+1 are issued before apply(s) on a double-buffered
    F ring: the PE never waits on the ACT exp stream.
  - everything downstream of PSUM is bf16 (output DRAM tensor too;
    host casts back to f32) -- halves DVE and output-DMA cost.
"""

import numpy as np

import concourse.bass as bass
import concourse.mybir as mybir
import concourse.tile as tile
from concourse import bacc, bass_utils
from concourse.bass import ts
from concourse.masks import make_identity

P = 128
C = 256          # feature channels
CQ = 64          # query channels
HW = 4096        # pixels per image
NB = HW // P     # 32 pixel blocks (contraction chunks)
SLICE = 512
NS = HW // SLICE  # 8 output column slices
NCORES = 8

F32 = mybir.dt.float32
BF16 = mybir.dt.bfloat16
EXP = mybir.ActivationFunctionType.Exp
COPY = mybir.ActivationFunctionType.Copy
AX_X = mybir.AxisListType.X


def _build_body(tc, src, ref, mask, wT, out, dbg=None):
    nc = tc.nc
    src_r = src.ap().rearrange("(ci p) j -> p ci j", p=P)   # [128, 2, 4096]
    ref_r = ref.ap().rearrange("(ci p) j -> p ci j", p=P)
    wT_r = wT.ap().rearrange("(ci p) o -> p ci o", p=P)     # [128, 2, 64]
    out_r = out.ap().rearrange("(cb p) j -> cb p j", p=P)   # [4, 128, 4096]

    with (
        tc.tile_pool(name="persist", bufs=1) as persist,
        tc.tile_pool(name="ps_s", bufs=4, space="PSUM") as ps_s,
        tc.tile_pool(name="ps_o", bufs=4, space="PSUM") as ps_o,
    ):
        # q duplicated into both partition halves so scores matmuls can be
        # row-packed: tile at rows 0-63 and rows 64-127 run concurrently.
        q2 = persist.tile([P, HW], BF16)
        pixT_src = persist.tile([P, NB, C], BF16)
        pixT_ref = persist.tile([P, NB, C], BF16)
        wT_sb = persist.tile([P, 2, CQ], BF16)
        zpart = persist.tile([P, NB, NS], F32)   # ACT accum of each exp
        z_all = persist.tile([P, NB], F32)
        invz = persist.tile([P, NB], F32)
        onem = persist.tile([P, HW], BF16)       # (1 - mask) broadcast
        m_rep = persist.tile([P, HW], BF16)      # mask broadcast
        refb = persist.tile([P, 2, HW], BF16)    # ref stays resident (blend)
        tmpf = persist.tile([P, HW], BF16)       # finalize scratch
        izb = persist.tile([P, HW], BF16)        # 1/Z broadcast
        o_sb = persist.tile([P, 4, HW], BF16)
        tmp7 = persist.tile([P, SLICE], BF16)
        exp_bias = persist.tile([P, 1], F32)
        ident = persist.tile([P, P], F32)
        identb = persist.tile([P, P], BF16)      # moving operand for PE transposes
        invz_T = persist.tile([NB, P], F32)
        ones_st = persist.tile([1, P], BF16)     # stationary ones row (K=1)
        warm_sb = persist.tile([P, SLICE], BF16) # zeroed filler operand
        mask_sb = persist.tile([1, HW], BF16)    # mask as a single row
        zrowb = persist.tile([1, HW], BF16)      # 1/Z as a single row
        nc.vector.memset(exp_bias, -64.0)
        nc.vector.memset(ones_st, 1.0)
        make_identity(nc, ident)
        make_identity(nc, identb)


        with tc.tile_pool(name="early", bufs=1) as early:
            srcb = early.tile([P, 2, HW], BF16)

            # PE warmup: back-to-back matmuls on zeroed data keep the HAM
            # clock gate at 8/8 while input DMAs stream in.  warm_fill is
            # re-used at every spot where the PE would otherwise idle for
            # >1us waiting on a DMA -- a short utilization dip makes the
            # HAM drop to 4/8 and the re-grant takes ~85us.
            # non-trivial warm data: zero operands toggle no bits in the
            # PE, which the HAM's power estimate can read as idle
            nc.gpsimd.iota(
                warm_sb, [[1, SLICE]], base=1, channel_multiplier=3,
                allow_small_or_imprecise_dtypes=True,
            )

            def warm_fill(n):
                wp = ps_o.tile([P, SLICE], F32, name="warm", tag="pso")
                for _ in range(n):
                    nc.tensor.matmul(
                        wp, warm_sb[:, 0:P], warm_sb, start=True, stop=True
                    )

            warm_fill(16)

            # DMA order matters twice over: (a) cast lines must stay at
            # 8KB (16KB corrupts), (b) the DMA subsystem keeps only ~4
            # outstanding DMA instructions, recycling slots by completion
            # -- so the bulk input casts go back-to-back with nothing
            # data-gated between them, wT (needed by conv) leads, and the
            # mask row trails.  pixT is produced on the PE below, not by
            # XBAR transposes, so nothing else competes for the fabric.
            nc.sync.dma_start(out=wT_sb, in_=wT_r)
            JH = HW // 2
            for h in range(2):
                jh = slice(h * JH, (h + 1) * JH)
                for ci in range(2):
                    nc.gpsimd.dma_start(out=srcb[:, ci, jh], in_=src_r[:, ci, jh])
            for h in range(2):
                jh = slice(h * JH, (h + 1) * JH)
                for ci in range(2):
                    nc.gpsimd.dma_start(out=refb[:, ci, jh], in_=ref_r[:, ci, jh])
            for h in range(2):
                jh = slice(h * JH, (h + 1) * JH)
                nc.gpsimd.dma_start(
                    out=mask_sb[:, jh],
                    in_=mask.ap()[jh].partition_broadcast(1),
                )

            # 1x1 conv: q = wT.T @ src_pix; q into both partition halves
            for s in range(NS):
                if s == 4:
                    warm_fill(14)
                sl = ts(s, SLICE)
                psq = ps_s.tile([CQ, SLICE], F32, name="psq", tag="pss")
                for ci in range(2):
                    nc.tensor.matmul(
                        psq,
                        wT_sb[:, ci, :],
                        srcb[:, ci, sl],
                        start=(ci == 0),
                        stop=(ci == 1),
                    )
                nc.vector.tensor_copy(out=q2[0:CQ, sl], in_=psq)
                nc.vector.tensor_copy(out=q2[CQ:P, sl], in_=psq)
            warm_fill(4)

            # pixT[j, c] = pix[c, j] via matmul(lhsT=pix-block, rhs=I):
            # out[j, c'] = sum_c pix[c, j] I[c, c'].  4 j-blocks share
            # one PSUM bank; one DVE copy drains all 4.
            for ci in range(2):
                cs = slice(ci * P, (ci + 1) * P)
                for g in range(NB // 4):
                    psT = ps_s.tile([P, 4, P], F32, name="psT", tag="pss")
                    for q in range(4):
                        jb = g * 4 + q
                        nc.tensor.matmul(
                            psT[:, q, :], srcb[:, ci, ts(jb, P)],
                            identb, start=True, stop=True,
                        )
                    nc.vector.tensor_copy(
                        out=pixT_src[:, g * 4 : g * 4 + 4, cs], in_=psT
                    )

        def scores_pair(s, f_sb, jp):
            sl = ts(s, SLICE)
            jb0, jb1 = 2 * jp, 2 * jp + 1
            pss0 = ps_s.tile([P, SLICE], F32, name="pss0", tag="pss")
            pss1 = ps_s.tile([P, SLICE], F32, name="pss1", tag="pss")
            nc.tensor.matmul(
                pss0, q2[0:CQ, ts(jb0, P)], q2[0:CQ, sl],
                start=True, stop=True, tile_position=(0, 0),
            )
            nc.tensor.matmul(
                pss1, q2[CQ:P, ts(jb1, P)], q2[CQ:P, sl],
                start=True, stop=True, tile_position=(CQ, 0),
            )
            for jb, pss in ((jb0, pss0), (jb1, pss1)):
                nc.scalar.activation(
                    out=f_sb[:, jb, :], in_=pss, func=EXP, bias=exp_bias,
                    accum_out=zpart[:, jb, s : s + 1],
                )

        def scores_and_exp(s, f_sb):
            for jp in range(NB // 2):
                scores_pair(s, f_sb, jp)

        def apply_mm(s, f_sb, mid_hook=None, next_scores=None):
            # jb-major; the NEXT slice's scores pairs interleave into the
            # stream one pair per two j-blocks (8 matmuls ~ 1.8us), which
            # matches the ACT exp drain rate -- the scores phase then
            # costs no standalone PE time and the 4-deep pss pool never
            # blocks.  jb-major keeps the f-ring WAR in lockstep: the
            # pair for j-blocks (2k, 2k+1) lands right after this slice's
            # apply has consumed those same blocks.
            psos = [
                ps_o.tile([P, SLICE], F32, name=f"pso{cb}", tag="pso")
                for cb in range(4)
            ]
            for jb in range(NB):
                if jb == 10 and mid_hook is not None:
                    mid_hook()
                if next_scores is not None and jb % 2 == 0 and jb >= 2:
                    scores_pair(*next_scores, jb // 2 - 1)
                for cb in range(4):
                    pt = pixT_src if cb < 2 else pixT_ref
                    lhs = pt[:, jb, (cb % 2) * P : (cb % 2 + 1) * P]
                    nc.tensor.matmul(
                        psos[cb], lhs, f_sb[:, jb, :],
                        start=(jb == 0), stop=(jb == NB - 1),
                    )
            if next_scores is not None:
                scores_pair(*next_scores, NB // 2 - 1)
            return psos

        def copy_out(s, psos):
            # src_att: plain PSUM->SBUF copy; ref_att: fold (1-m) in
            sl = ts(s, SLICE)
            nc.vector.tensor_copy(out=o_sb[:, 0, sl], in_=psos[0])
            nc.vector.tensor_copy(out=o_sb[:, 1, sl], in_=psos[1])
            nc.vector.tensor_mul(o_sb[:, 2, sl], psos[2], onem[:, sl])
            nc.vector.tensor_mul(o_sb[:, 3, sl], psos[3], onem[:, sl])

        def izb_broadcast():
            # 1/Z row -> [128, HW] via ones-vector matmuls; ACT drains PSUM
            # (the DVE is busy with finalize at this point)
            for s2 in range(NS):
                sl2 = ts(s2, SLICE)
                psz = ps_s.tile([P, SLICE], F32, name="psz", tag="pss")
                nc.tensor.matmul(
                    psz, ones_st, zrowb[:, sl2], start=True, stop=True
                )
                nc.scalar.activation(out=izb[:, sl2], in_=psz, func=COPY)

        def finalize(lo, hi, dma_engines):
            """Normalize + blend + store for pixel columns [lo:hi)."""
            r = slice(lo, hi)
            for ci in range(2):
                nc.vector.tensor_mul(o_sb[:, 2 + ci, r], o_sb[:, 2 + ci, r], izb[:, r])
                nc.vector.tensor_mul(tmpf[:, r], m_rep[:, r], refb[:, ci, r])
                nc.vector.tensor_add(o_sb[:, 2 + ci, r], o_sb[:, 2 + ci, r], tmpf[:, r])
                nc.vector.tensor_mul(o_sb[:, ci, r], o_sb[:, ci, r], izb[:, r])
            # out rows: [flow(=cb2,3), src_att(=cb0,1)]
            for k, cb in enumerate([2, 3, 0, 1]):
                eng = dma_engines[k % len(dma_engines)]
                eng.dma_start(out=out_r[k, :, r], in_=o_sb[:, cb, r])

        with tc.tile_pool(name="fbuf", bufs=2) as fbuf:
            # double-buffered F ring: exp(s+1) writes one buffer while
            # apply(s) streams the other, so scores stay one slice ahead.
            # The ref-side PE transposes and the mask broadcast matmuls
            # slot in after scores(0)/scores(1) -- by then refb / mask_sb
            # have landed, so the PE never stalls on them.
            fbufs = [
                fbuf.tile([P, NB, SLICE], BF16, name="f_sb", tag="f")
                for _ in range(2)
            ]
            # scores(0) and scores(1) pairs are ACT-drain-paced (1.4us
            # per pair vs 0.46us of PE work); the ref-side PE transposes
            # interleave into the stall slots -- one 4-block group per
            # two pairs -- so the PE stays dense and no standalone T_ref
            # phase remains.
            def tref_group(ci, g):
                cs = slice(ci * P, (ci + 1) * P)
                psT = ps_s.tile([P, 4, P], F32, name="psT", tag="pss")
                for q in range(4):
                    jb = g * 4 + q
                    nc.tensor.matmul(
                        psT[:, q, :], refb[:, ci, ts(jb, P)],
                        identb, start=True, stop=True,
                    )
                nc.vector.tensor_copy(
                    out=pixT_ref[:, g * 4 : g * 4 + 4, cs], in_=psT
                )

            tgroups = [(ci, g) for ci in range(2) for g in range(NB // 4)]
            ti = 0
            for k in range(2 * (NB // 2)):
                s01, jp = divmod(k, NB // 2)
                scores_pair(s01, fbufs[s01], jp)
                if k % 2 == 1 and ti < len(tgroups):
                    tref_group(*tgroups[ti])
                    ti += 1
            while ti < len(tgroups):
                tref_group(*tgroups[ti])
                ti += 1
            # mask column-broadcast + drains (DVE: m and 1-m)
            for s in range(NS):
                sl = ts(s, SLICE)
                psm = ps_s.tile([P, SLICE], F32, name="psm", tag="pss")
                nc.tensor.matmul(
                    psm, ones_st, mask_sb[:, sl], start=True, stop=True
                )
                nc.vector.tensor_copy(out=m_rep[:, sl], in_=psm)
                nc.vector.tensor_scalar(
                    out=onem[:, sl], in0=psm, scalar1=-1.0, scalar2=1.0,
                    op0=mybir.AluOpType.mult, op1=mybir.AluOpType.add,
                )
            for s in range(NS - 2):
                psos = apply_mm(
                    s, fbufs[s % 2], next_scores=(s + 2, fbufs[s % 2])
                )
                copy_out(s, psos)
            s7 = NS - 1
            f_cur = fbufs[(NS - 2) % 2]
            f_sb = fbufs[s7 % 2]
            # Z partials all land once exp(s7) retires (during apply(s6));
            # issuing the DVE reduce before copy_out(s6) lets it run early
            nc.vector.reduce_sum(out=z_all, in_=zpart, axis=AX_X)
            nc.vector.reciprocal(out=invz, in_=z_all)
            psos = apply_mm(s7 - 1, f_cur)
            copy_out(s7 - 1, psos)
            ps_t = ps_s.tile([NB, P], F32, name="ps_t", tag="pss")
            nc.tensor.transpose(ps_t, invz[:, :], ident)
            nc.vector.tensor_copy(out=invz_T, in_=ps_t)
            # flatten [32 partitions, 128] -> one [1, 4096] row (SBUF->SBUF
            # DMA crosses partitions; f32 -> bf16 converts on the way)
            nc.gpsimd.dma_start(
                out=zrowb.rearrange("a (b q) -> a b q", q=P), in_=invz_T
            )
            # slice 7 apply; after 10 j-blocks the zrowb row has landed, so
            # the izb broadcast matmuls slot into the middle of the stream
            psos7 = apply_mm(s7, f_sb, mid_hook=izb_broadcast)
            finalize(0, (NS - 1) * SLICE, [nc.sync, nc.scalar, nc.gpsimd])
            # slice 7: copy-out doubles as normalize + blend
            sl7 = ts(s7, SLICE)
            nc.vector.tensor_mul(tmp7, onem[:, sl7], izb[:, sl7])
            nc.vector.tensor_mul(o_sb[:, 0, sl7], psos7[0], izb[:, sl7])
            nc.vector.tensor_mul(o_sb[:, 1, sl7], psos7[1], izb[:, sl7])
            for ci in range(2):
                nc.vector.tensor_mul(o_sb[:, 2 + ci, sl7], psos7[2 + ci], tmp7)
                nc.vector.tensor_mul(tmpf[:, sl7], m_rep[:, sl7], refb[:, ci, sl7])
                nc.vector.tensor_add(
                    o_sb[:, 2 + ci, sl7], o_sb[:, 2 + ci, sl7], tmpf[:, sl7]
                )
            for k, cb in enumerate([2, 3, 0, 1]):
                eng = [nc.sync, nc.scalar, nc.gpsimd, nc.sync][k]
                eng.dma_start(out=out_r[k, :, sl7], in_=o_sb[:, cb, sl7])

            if dbg is not None:
                nc.sync.dma_start(out=dbg["q2"].ap(), in_=q2)
                nc.sync.dma_start(
                    out=dbg["zpart"].ap().rearrange("p (b s) -> p b s", s=NS),
                    in_=zpart,
                )
                nc.sync.dma_start(out=dbg["invz"].ap(), in_=invz)
                nc.sync.dma_start(out=dbg["izb"].ap(), in_=izb)
                nc.sync.dma_start(out=dbg["onem"].ap(), in_=onem)

                nc.sync.dma_start(
                    out=dbg["f7"].ap().rearrange("p (b i) -> p b i", b=NB),
                    in_=f_sb,
                )


def build():
    nc = bacc.Bacc(
        "TRN2",
        target_bir_lowering=False,
        debug=False,
        enable_asserts=False,
        num_devices=NCORES,
    )
    src = nc.dram_tensor("src", (C, HW), F32, kind="ExternalInput")
    ref = nc.dram_tensor("ref", (C, HW), F32, kind="ExternalInput")
    mask = nc.dram_tensor("mask", (HW,), F32, kind="ExternalInput")
    wT = nc.dram_tensor("wT", (C, CQ), BF16, kind="ExternalInput")
    out = nc.dram_tensor("out", (2 * C, HW), BF16, kind="ExternalOutput")
    with tile.TileContext(nc) as tc:
        _build_body(tc, src, ref, mask, wT, out)
    nc.compile()
    return nc


_CACHE = {}


def _get_nc():
    if "nc" not in _CACHE:
        _CACHE["nc"] = build()
    return _CACHE["nc"]


def _in_maps(src_mask, src_feature, ref_feature, conv_w):
    import ml_dtypes

    n_batch = src_feature.shape[0]
    wT = np.ascontiguousarray(
        np.asarray(conv_w, dtype=np.float32).T.astype(ml_dtypes.bfloat16)
    )
    maps = []
    for n in range(n_batch):
        maps.append(
            {
                "src": np.ascontiguousarray(
                    np.asarray(src_feature[n], dtype=np.float32).reshape(C, HW)
                ),
                "ref": np.ascontiguousarray(
                    np.asarray(ref_feature[n], dtype=np.float32).reshape(C, HW)
                ),
                "mask": np.ascontiguousarray(
                    np.asarray(src_mask[n], dtype=np.float32).reshape(HW)
                ),
                "wT": wT,
            }
        )
    return maps


def _install_ntff_hook():
    """The agent image's antenv lacks axon_hooks; recreate it so
    run_bass_kernel_spmd(trace=True) can capture NTFF profiles."""
    import sys
    import types

    if "antenv.axon_hooks" in sys.modules:
        return
    import antenv
    from trn_agent_boot.trn_boot import _ntff_profile_via_ctypes

    hook = _ntff_profile_via_ctypes("/opt/axon/libaxon_pjrt.so")
    mod = types.ModuleType("antenv.axon_hooks")
    mod._hook = hook
    mod.set_axon_ntff_profile_hook = lambda h: setattr(mod, "_hook", h)
    mod.get_axon_ntff_profile_hook = lambda: mod._hook
    sys.modules["antenv.axon_hooks"] = mod
    antenv.axon_hooks = mod


def run(src_mask, src_feature, ref_feature, conv_w, trace=False):
    """Run on 8 NeuronCores. Returns (output [N,2C,H,W], BassKernelResults)."""
    n_batch, c, h, w = src_feature.shape
    if trace:
        _install_ntff_hook()
    nc = _get_nc()
    maps = _in_maps(src_mask, src_feature, ref_feature, conv_w)
    res = bass_utils.run_bass_kernel_spmd(
        nc, maps, core_ids=list(range(NCORES)), trace=trace
    )
    out = np.stack([np.asarray(r["out"]) for r in res.results], axis=0)
    return out.reshape(n_batch, 2 * c, h, w).astype(np.float32), res


def kernel(src_mask, src_feature, ref_feature, conv_w):
    out, _ = run(src_mask, src_feature, ref_feature, conv_w)
    return out



# revision 3
# speedup vs baseline: 1.0732x; 1.0732x over previous
"""Trainium2 Bass kernel for ExampleGuidedAttention (N=8, C=256, H=W=64).

Data-parallel over batch N across 8 NeuronCores; each core computes one
batch element's full guided attention.

Algorithm notes (per core):
  q = conv_w @ src_pix                      [64, 4096]   (PE, bf16)
  S^T[j,i] = sum_o q[o,j] q[o,i]            (PE, bf16; S symmetric; two
             j-blocks packed in the 128x128 array via tile_position
             (row groups 0-63 / 64-127) since the contraction is only 64)
  F[j,i] = exp(S^T[j,i] - 64)               (ACT; global shift keeps fp32
             exp in range -- softmax ratio unchanged; diag scores are
             chi2(64) so they reach ~120).  The ACT accumulator
             (accum_out) yields Z partials for free: Z[j] = sum_i F[j,i]
             equals the softmax denominator because S is symmetric.
  O[c,i] = sum_j pixT[j,c] * F[j,i]         (PE, bf16, natural layout)
  out    = [ (1-m)*ref_att*invZ + m*ref ; src_att*invZ ]

Performance structure (vs the v1 kernel):
  - inputs land as 8KB-contiguous partition lines ([128, 2048] convert
    chunks): ~2x the bandwidth of the 2KB-line version.  NOTE: 16KB
    lines silently corrupt (descriptor length-field limit) -- keep
    convert-DMA lines at <= 8KB.
  - all [128, HW] column-broadcasts (mask, 1/Z) are ones-vector matmuls
    on the PE into PSUM -- the partition_broadcast DMA path is
    software-dynamic and costs ~25us per 2MB.
  - Z comes free from the exp pass via the ACT accumulator.
  - finalize is restructured as  flow = [(1-m)*ra]*izb + m*ref  with
    (1-m) folded into the PSUM->SBUF copy-out of each slice and m*ref
    precomputed during the input phase, so the tail after the last
    apply matmul is only slice 7's small DVE ops + 4 small DMAs.
  - scores for slice s+1 are issued before apply(s) on a double-buffered
    F ring: the PE never waits on the ACT exp stream.
  - everything downstream of PSUM is bf16 (output DRAM tensor too;
    host casts back to f32) -- halves DVE and output-DMA cost.
"""

import numpy as np

import concourse.bass as bass
import concourse.mybir as mybir
import concourse.tile as tile
from concourse import bacc, bass_utils
from concourse.bass import ts
from concourse.masks import make_identity

P = 128
C = 256          # feature channels
CQ = 64          # query channels
HW = 4096        # pixels per image
NB = HW // P     # 32 pixel blocks (contraction chunks)
SLICE = 512
NS = HW // SLICE  # 8 output column slices
NCORES = 8

F32 = mybir.dt.float32
BF16 = mybir.dt.bfloat16
EXP = mybir.ActivationFunctionType.Exp
COPY = mybir.ActivationFunctionType.Copy
AX_X = mybir.AxisListType.X


def _build_body(tc, src, ref, mask, wT, out, dbg=None):
    nc = tc.nc
    src_r = src.ap().rearrange("(ci p) j -> p ci j", p=P)   # [128, 2, 4096]
    ref_r = ref.ap().rearrange("(ci p) j -> p ci j", p=P)
    wT_r = wT.ap().rearrange("(ci p) o -> p ci o", p=P)     # [128, 2, 64]
    out_r = out.ap().rearrange("(cb p) j -> cb p j", p=P)   # [4, 128, 4096]

    with (
        tc.tile_pool(name="persist", bufs=1) as persist,
        tc.tile_pool(name="ps_s", bufs=2, space="PSUM") as ps_s,
        tc.tile_pool(name="ps_o", bufs=4, space="PSUM") as ps_o,
    ):
        # q duplicated into both partition halves so scores matmuls can be
        # row-packed: tile at rows 0-63 and rows 64-127 run concurrently.
        q2 = persist.tile([P, HW], BF16)
        pixT_src = persist.tile([P, NB, C], BF16)
        pixT_ref = persist.tile([P, NB, C], BF16)
        wT_sb = persist.tile([P, 2, CQ], BF16)
        zpart = persist.tile([P, NB, NS], F32)   # ACT accum of each exp
        z_all = persist.tile([P, NB], F32)
        invz = persist.tile([P, NB], F32)
        onem = persist.tile([P, HW], BF16)       # (1 - mask) broadcast
        m_rep = persist.tile([P, HW], BF16)      # mask broadcast
        refb = persist.tile([P, 2, HW], BF16)    # ref stays resident (blend)
        tmpf = persist.tile([P, HW], BF16)       # finalize scratch
        izb = persist.tile([P, HW], BF16)        # 1/Z broadcast
        o_sb = persist.tile([P, 4, HW], BF16)
        tmp7 = persist.tile([P, SLICE], BF16)
        exp_bias = persist.tile([P, 1], F32)
        ident = persist.tile([P, P], F32)
        identb = persist.tile([P, P], BF16)      # moving operand for PE transposes
        invz_T = persist.tile([NB, P], F32)
        ones_st = persist.tile([1, P], BF16)     # stationary ones row (K=1)
        warm_sb = persist.tile([P, SLICE], BF16) # zeroed filler operand
        mask_sb = persist.tile([1, HW], BF16)    # mask as a single row
        zrowb = persist.tile([1, HW], BF16)      # 1/Z as a single row
        nc.vector.memset(exp_bias, -64.0)
        nc.vector.memset(ones_st, 1.0)
        make_identity(nc, ident)
        make_identity(nc, identb)


        with tc.tile_pool(name="early", bufs=1) as early:
            srcb = early.tile([P, 2, HW], BF16)

            # PE warmup: back-to-back matmuls on zeroed data keep the HAM
            # clock gate at 8/8 while input DMAs stream in.  warm_fill is
            # re-used at every spot where the PE would otherwise idle for
            # >1us waiting on a DMA -- a short utilization dip makes the
            # HAM drop to 4/8 and the re-grant takes ~85us.
            # non-trivial warm data: zero operands toggle no bits in the
            # PE, which the HAM's power estimate can read as idle
            nc.gpsimd.iota(
                warm_sb, [[1, SLICE]], base=1, channel_multiplier=3,
                allow_small_or_imprecise_dtypes=True,
            )

            def warm_fill(n):
                wp = ps_o.tile([P, SLICE], F32, name="warm", tag="pso")
                for _ in range(n):
                    nc.tensor.matmul(
                        wp, warm_sb[:, 0:P], warm_sb, start=True, stop=True
                    )

            warm_fill(16)

            # DMA order matters twice over: (a) cast lines must stay at
            # 8KB (16KB corrupts), (b) the DMA subsystem keeps only ~4
            # outstanding DMA instructions, recycling slots by completion
            # -- so the bulk input casts go back-to-back with nothing
            # data-gated between them, wT (needed by conv) leads, and the
            # mask row trails.  pixT is produced on the PE below, not by
            # XBAR transposes, so nothing else competes for the fabric.
            nc.sync.dma_start(out=wT_sb, in_=wT_r)
            JH = HW // 2
            for h in range(2):
                jh = slice(h * JH, (h + 1) * JH)
                for ci in range(2):
                    nc.gpsimd.dma_start(out=srcb[:, ci, jh], in_=src_r[:, ci, jh])
            for h in range(2):
                jh = slice(h * JH, (h + 1) * JH)
                for ci in range(2):
                    nc.gpsimd.dma_start(out=refb[:, ci, jh], in_=ref_r[:, ci, jh])
            for h in range(2):
                jh = slice(h * JH, (h + 1) * JH)
                nc.gpsimd.dma_start(
                    out=mask_sb[:, jh],
                    in_=mask.ap()[jh].partition_broadcast(1),
                )

            # 1x1 conv: q = wT.T @ src_pix; q into both partition halves
            for s in range(NS):
                if s == 4:
                    warm_fill(14)
                sl = ts(s, SLICE)
                psq = ps_s.tile([CQ, SLICE], F32, name="psq", tag="pss")
                for ci in range(2):
                    nc.tensor.matmul(
                        psq,
                        wT_sb[:, ci, :],
                        srcb[:, ci, sl],
                        start=(ci == 0),
                        stop=(ci == 1),
                    )
                nc.vector.tensor_copy(out=q2[0:CQ, sl], in_=psq)
                nc.vector.tensor_copy(out=q2[CQ:P, sl], in_=psq)
            warm_fill(4)

            # pixT[j, c] = pix[c, j] via matmul(lhsT=pix-block, rhs=I):
            # out[j, c'] = sum_c pix[c, j] I[c, c'].  4 j-blocks share
            # one PSUM bank; one DVE copy drains all 4.
            for ci in range(2):
                cs = slice(ci * P, (ci + 1) * P)
                for g in range(NB // 4):
                    psT = ps_s.tile([P, 4, P], F32, name="psT", tag="pss")
                    for q in range(4):
                        jb = g * 4 + q
                        nc.tensor.matmul(
                            psT[:, q, :], srcb[:, ci, ts(jb, P)],
                            identb, start=True, stop=True,
                        )
                    nc.vector.tensor_copy(
                        out=pixT_src[:, g * 4 : g * 4 + 4, cs], in_=psT
                    )

        def scores_pair(s, f_sb, jp):
            # pair psum tile spans 2 banks -> ONE [128,1024] ACTIVATE drains
            # both j-blocks ((N+352)/1.2ns: 1147 vs 2x720) and the Z row-sums
            # move to the idle DVE (per-pair reduce) instead of the ACT
            # accumulator reads (294ns each) -- the ACT stream stops gating
            # the PE's scores pairs.
            sl = ts(s, SLICE)
            jb0, jb1 = 2 * jp, 2 * jp + 1
            pss = ps_s.tile([P, 2, SLICE], F32, name="pss", tag="pss")
            nc.tensor.matmul(
                pss[:, 0, :], q2[0:CQ, ts(jb0, P)], q2[0:CQ, sl],
                start=True, stop=True, tile_position=(0, 0),
            )
            nc.tensor.matmul(
                pss[:, 1, :], q2[CQ:P, ts(jb1, P)], q2[CQ:P, sl],
                start=True, stop=True, tile_position=(CQ, 0),
            )
            nc.scalar.activation(
                out=f_sb[:, jb0 : jb0 + 2, :], in_=pss, func=EXP,
                bias=exp_bias,
            )
            nc.vector.reduce_sum(
                out=zpart[:, jb0 : jb0 + 2, s : s + 1],
                in_=f_sb[:, jb0 : jb0 + 2, :],
                axis=AX_X,
            )

        def scores_and_exp(s, f_sb):
            for jp in range(NB // 2):
                scores_pair(s, f_sb, jp)

        def apply_mm(s, f_sb, mid_hook=None, next_scores=None):
            # jb-major; the NEXT slice's scores pairs interleave into the
            # stream one pair per two j-blocks (8 matmuls ~ 1.8us), which
            # matches the ACT exp drain rate -- the scores phase then
            # costs no standalone PE time and the 4-deep pss pool never
            # blocks.  jb-major keeps the f-ring WAR in lockstep: the
            # pair for j-blocks (2k, 2k+1) lands right after this slice's
            # apply has consumed those same blocks.
            psos = [
                ps_o.tile([P, SLICE], F32, name=f"pso{cb}", tag="pso")
                for cb in range(4)
            ]
            for jb in range(NB):
                if jb == 10 and mid_hook is not None:
                    mid_hook()
                if next_scores is not None and jb % 2 == 0 and jb >= 2:
                    scores_pair(*next_scores, jb // 2 - 1)
                for cb in range(4):
                    pt = pixT_src if cb < 2 else pixT_ref
                    lhs = pt[:, jb, (cb % 2) * P : (cb % 2 + 1) * P]
                    nc.tensor.matmul(
                        psos[cb], lhs, f_sb[:, jb, :],
                        start=(jb == 0), stop=(jb == NB - 1),
                    )
            if next_scores is not None:
                scores_pair(*next_scores, NB // 2 - 1)
            return psos

        def copy_out(s, psos):
            # src_att: plain PSUM->SBUF copy; ref_att: fold (1-m) in
            sl = ts(s, SLICE)
            nc.vector.tensor_copy(out=o_sb[:, 0, sl], in_=psos[0])
            nc.vector.tensor_copy(out=o_sb[:, 1, sl], in_=psos[1])
            nc.vector.tensor_mul(o_sb[:, 2, sl], psos[2], onem[:, sl])
            nc.vector.tensor_mul(o_sb[:, 3, sl], psos[3], onem[:, sl])

        def izb_broadcast():
            # 1/Z row -> [128, HW] via ones-vector matmuls; ACT drains PSUM
            # (the DVE is busy with finalize at this point)
            for s2 in range(NS):
                sl2 = ts(s2, SLICE)
                psz = ps_s.tile([P, SLICE], F32, name="psz", tag="pss")
                nc.tensor.matmul(
                    psz, ones_st, zrowb[:, sl2], start=True, stop=True
                )
                nc.scalar.activation(out=izb[:, sl2], in_=psz, func=COPY)

        def finalize(lo, hi, dma_engines):
            """Normalize + blend + store for pixel columns [lo:hi)."""
            r = slice(lo, hi)
            for ci in range(2):
                nc.vector.tensor_mul(o_sb[:, 2 + ci, r], o_sb[:, 2 + ci, r], izb[:, r])
                nc.vector.tensor_mul(tmpf[:, r], m_rep[:, r], refb[:, ci, r])
                nc.vector.tensor_add(o_sb[:, 2 + ci, r], o_sb[:, 2 + ci, r], tmpf[:, r])
                nc.vector.tensor_mul(o_sb[:, ci, r], o_sb[:, ci, r], izb[:, r])
            # out rows: [flow(=cb2,3), src_att(=cb0,1)]
            for k, cb in enumerate([2, 3, 0, 1]):
                eng = dma_engines[k % len(dma_engines)]
                eng.dma_start(out=out_r[k, :, r], in_=o_sb[:, cb, r])

        with tc.tile_pool(name="fbuf", bufs=2) as fbuf:
            # double-buffered F ring: exp(s+1) writes one buffer while
            # apply(s) streams the other, so scores stay one slice ahead.
            # The ref-side PE transposes and the mask broadcast matmuls
            # slot in after scores(0)/scores(1) -- by then refb / mask_sb
            # have landed, so the PE never stalls on them.
            fbufs = [
                fbuf.tile([P, NB, SLICE], BF16, name="f_sb", tag="f")
                for _ in range(2)
            ]
            # scores(0) and scores(1) pairs are ACT-drain-paced (1.4us
            # per pair vs 0.46us of PE work); the ref-side PE transposes
            # interleave into the stall slots -- one 4-block group per
            # two pairs -- so the PE stays dense and no standalone T_ref
            # phase remains.
            def tref_group(ci, g):
                cs = slice(ci * P, (ci + 1) * P)
                psT = ps_s.tile([P, 4, P], F32, name="psT", tag="pss")
                for q in range(4):
                    jb = g * 4 + q
                    nc.tensor.matmul(
                        psT[:, q, :], refb[:, ci, ts(jb, P)],
                        identb, start=True, stop=True,
                    )
                nc.vector.tensor_copy(
                    out=pixT_ref[:, g * 4 : g * 4 + 4, cs], in_=psT
                )

            tgroups = [(ci, g) for ci in range(2) for g in range(NB // 4)]
            ti = 0
            for k in range(2 * (NB // 2)):
                s01, jp = divmod(k, NB // 2)
                scores_pair(s01, fbufs[s01], jp)
                if k % 2 == 1 and ti < len(tgroups):
                    tref_group(*tgroups[ti])
                    ti += 1
            while ti < len(tgroups):
                tref_group(*tgroups[ti])
                ti += 1
            # mask column-broadcast + drains (DVE: m and 1-m)
            for s in range(NS):
                sl = ts(s, SLICE)
                psm = ps_s.tile([P, SLICE], F32, name="psm", tag="pss")
                nc.tensor.matmul(
                    psm, ones_st, mask_sb[:, sl], start=True, stop=True
                )
                nc.vector.tensor_copy(out=m_rep[:, sl], in_=psm)
                nc.vector.tensor_scalar(
                    out=onem[:, sl], in0=psm, scalar1=-1.0, scalar2=1.0,
                    op0=mybir.AluOpType.mult, op1=mybir.AluOpType.add,
                )
            for s in range(NS - 2):
                psos = apply_mm(
                    s, fbufs[s % 2], next_scores=(s + 2, fbufs[s % 2])
                )
                copy_out(s, psos)
            s7 = NS - 1
            f_cur = fbufs[(NS - 2) % 2]
            f_sb = fbufs[s7 % 2]
            # Z partials all land once exp(s7) retires (during apply(s6));
            # issuing the DVE reduce before copy_out(s6) lets it run early
            nc.vector.reduce_sum(out=z_all, in_=zpart, axis=AX_X)
            nc.vector.reciprocal(out=invz, in_=z_all)
            psos = apply_mm(s7 - 1, f_cur)
            copy_out(s7 - 1, psos)
            ps_t = ps_s.tile([NB, P], F32, name="ps_t", tag="pss")
            nc.tensor.transpose(ps_t, invz[:, :], ident)
            nc.vector.tensor_copy(out=invz_T, in_=ps_t)
            # flatten [32 partitions, 128] -> one [1, 4096] row (SBUF->SBUF
            # DMA crosses partitions; f32 -> bf16 converts on the way)
            nc.gpsimd.dma_start(
                out=zrowb.rearrange("a (b q) -> a b q", q=P), in_=invz_T
            )
            # slice 7 apply; after 10 j-blocks the zrowb row has landed, so
            # the izb broadcast matmuls slot into the middle of the stream
            psos7 = apply_mm(s7, f_sb, mid_hook=izb_broadcast)
            finalize(0, (NS - 1) * SLICE, [nc.sync, nc.scalar, nc.gpsimd])
            # slice 7: copy-out doubles as normalize + blend
            sl7 = ts(s7, SLICE)
            nc.vector.tensor_mul(tmp7, onem[:, sl7], izb[:, sl7])
            nc.vector.tensor_mul(o_sb[:, 0, sl7], psos7[0], izb[:, sl7])
            nc.vector.tensor_mul(o_sb[:, 1, sl7], psos7[1], izb[:, sl7])
            for ci in range(2):
                nc.vector.tensor_mul(o_sb[:, 2 + ci, sl7], psos7[2 + ci], tmp7)
                nc.vector.tensor_mul(tmpf[:, sl7], m_rep[:, sl7], refb[:, ci, sl7])
                nc.vector.tensor_add(
                    o_sb[:, 2 + ci, sl7], o_sb[:, 2 + ci, sl7], tmpf[:, sl7]
                )
            for k, cb in enumerate([2, 3, 0, 1]):
                eng = [nc.sync, nc.scalar, nc.gpsimd, nc.sync][k]
                eng.dma_start(out=out_r[k, :, sl7], in_=o_sb[:, cb, sl7])

            if dbg is not None:
                nc.sync.dma_start(out=dbg["q2"].ap(), in_=q2)
                nc.sync.dma_start(
                    out=dbg["zpart"].ap().rearrange("p (b s) -> p b s", s=NS),
                    in_=zpart,
                )
                nc.sync.dma_start(out=dbg["invz"].ap(), in_=invz)
                nc.sync.dma_start(out=dbg["izb"].ap(), in_=izb)
                nc.sync.dma_start(out=dbg["onem"].ap(), in_=onem)

                nc.sync.dma_start(
                    out=dbg["f7"].ap().rearrange("p (b i) -> p b i", b=NB),
                    in_=f_sb,
                )


def build():
    nc = bacc.Bacc(
        "TRN2",
        target_bir_lowering=False,
        debug=False,
        enable_asserts=False,
        num_devices=NCORES,
    )
    src = nc.dram_tensor("src", (C, HW), F32, kind="ExternalInput")
    ref = nc.dram_tensor("ref", (C, HW), F32, kind="ExternalInput")
    mask = nc.dram_tensor("mask", (HW,), F32, kind="ExternalInput")
    wT = nc.dram_tensor("wT", (C, CQ), BF16, kind="ExternalInput")
    out = nc.dram_tensor("out", (2 * C, HW), BF16, kind="ExternalOutput")
    with tile.TileContext(nc) as tc:
        _build_body(tc, src, ref, mask, wT, out)
    nc.compile()
    return nc


_CACHE = {}


def _get_nc():
    if "nc" not in _CACHE:
        _CACHE["nc"] = build()
    return _CACHE["nc"]


def _in_maps(src_mask, src_feature, ref_feature, conv_w):
    import ml_dtypes

    n_batch = src_feature.shape[0]
    wT = np.ascontiguousarray(
        np.asarray(conv_w, dtype=np.float32).T.astype(ml_dtypes.bfloat16)
    )
    maps = []
    for n in range(n_batch):
        maps.append(
            {
                "src": np.ascontiguousarray(
                    np.asarray(src_feature[n], dtype=np.float32).reshape(C, HW)
                ),
                "ref": np.ascontiguousarray(
                    np.asarray(ref_feature[n], dtype=np.float32).reshape(C, HW)
                ),
                "mask": np.ascontiguousarray(
                    np.asarray(src_mask[n], dtype=np.float32).reshape(HW)
                ),
                "wT": wT,
            }
        )
    return maps


def _install_ntff_hook():
    """The agent image's antenv lacks axon_hooks; recreate it so
    run_bass_kernel_spmd(trace=True) can capture NTFF profiles."""
    import sys
    import types

    if "antenv.axon_hooks" in sys.modules:
        return
    import antenv
    from trn_agent_boot.trn_boot import _ntff_profile_via_ctypes

    hook = _ntff_profile_via_ctypes("/opt/axon/libaxon_pjrt.so")
    mod = types.ModuleType("antenv.axon_hooks")
    mod._hook = hook
    mod.set_axon_ntff_profile_hook = lambda h: setattr(mod, "_hook", h)
    mod.get_axon_ntff_profile_hook = lambda: mod._hook
    sys.modules["antenv.axon_hooks"] = mod
    antenv.axon_hooks = mod


def run(src_mask, src_feature, ref_feature, conv_w, trace=False):
    """Run on 8 NeuronCores. Returns (output [N,2C,H,W], BassKernelResults)."""
    n_batch, c, h, w = src_feature.shape
    if trace:
        _install_ntff_hook()
    nc = _get_nc()
    maps = _in_maps(src_mask, src_feature, ref_feature, conv_w)
    res = bass_utils.run_bass_kernel_spmd(
        nc, maps, core_ids=list(range(NCORES)), trace=trace
    )
    out = np.stack([np.asarray(r["out"]) for r in res.results], axis=0)
    return out.reshape(n_batch, 2 * c, h, w).astype(np.float32), res


def kernel(src_mask, src_feature, ref_feature, conv_w):
    out, _ = run(src_mask, src_feature, ref_feature, conv_w)
    return out



# revision 12
# speedup vs baseline: 1.0826x; 1.0087x over previous
"""Trainium2 Bass kernel for ExampleGuidedAttention (N=8, C=256, H=W=64).

Data-parallel over batch N across 8 NeuronCores; each core computes one
batch element's full guided attention.

Algorithm notes (per core):
  q = conv_w @ src_pix                      [64, 4096]   (PE, bf16)
  S^T[j,i] = sum_o q[o,j] q[o,i]            (PE, bf16; S symmetric; two
             j-blocks packed in the 128x128 array via tile_position
             (row groups 0-63 / 64-127) since the contraction is only 64)
  F[j,i] = exp(S^T[j,i] - 64)               (ACT; global shift keeps fp32
             exp in range -- softmax ratio unchanged; diag scores are
             chi2(64) so they reach ~120).  The ACT accumulator
             (accum_out) yields Z partials for free: Z[j] = sum_i F[j,i]
             equals the softmax denominator because S is symmetric.
  O[c,i] = sum_j pixT[j,c] * F[j,i]         (PE, bf16, natural layout)
  out    = [ (1-m)*ref_att*invZ + m*ref ; src_att*invZ ]

Performance structure (vs the v1 kernel):
  - inputs land as 8KB-contiguous partition lines ([128, 2048] convert
    chunks): ~2x the bandwidth of the 2KB-line version.  NOTE: 16KB
    lines silently corrupt (descriptor length-field limit) -- keep
    convert-DMA lines at <= 8KB.
  - all [128, HW] column-broadcasts (mask, 1/Z) are ones-vector matmuls
    on the PE into PSUM -- the partition_broadcast DMA path is
    software-dynamic and costs ~25us per 2MB.
  - Z comes free from the exp pass via the ACT accumulator.
  - finalize is restructured as  flow = [(1-m)*ra]*izb + m*ref  with
    (1-m) folded into the PSUM->SBUF copy-out of each slice and m*ref
    precomputed during the input phase, so the tail after the last
    apply matmul is only slice 7's small DVE ops + 4 small DMAs.
  - scores for slice s+1 are issued before apply(s) on a double-buffered
    F ring: the PE never waits on the ACT exp stream.
  - everything downstream of PSUM is bf16 (output DRAM tensor too;
    host casts back to f32) -- halves DVE and output-DMA cost.
"""

import numpy as np

import concourse.bass as bass
import concourse.mybir as mybir
import concourse.tile as tile
from concourse import bacc, bass_utils
from concourse.bass import ts
from concourse.masks import make_identity

P = 128
C = 256          # feature channels
CQ = 64          # query channels
HW = 4096        # pixels per image
NB = HW // P     # 32 pixel blocks (contraction chunks)
SLICE = 512
NS = HW // SLICE  # 8 output column slices
NCORES = 8

F32 = mybir.dt.float32
BF16 = mybir.dt.bfloat16
EXP = mybir.ActivationFunctionType.Exp
COPY = mybir.ActivationFunctionType.Copy
AX_X = mybir.AxisListType.X


def _build_body(tc, src, ref, mask, wT, out, dbg=None):
    nc = tc.nc
    src_r = src.ap().rearrange("(ci p) j -> p ci j", p=P)   # [128, 2, 4096]
    ref_r = ref.ap().rearrange("(ci p) j -> p ci j", p=P)
    wT_r = wT.ap().rearrange("(ci p) o -> p ci o", p=P)     # [128, 2, 64]
    out_r = out.ap().rearrange("(cb p) j -> cb p j", p=P)   # [4, 128, 4096]

    with (
        tc.tile_pool(name="persist", bufs=1) as persist,
        tc.tile_pool(name="ps_s", bufs=2, space="PSUM") as ps_s,
        tc.tile_pool(name="ps_o", bufs=4, space="PSUM") as ps_o,
    ):
        # q duplicated into both partition halves so scores matmuls can be
        # row-packed: tile at rows 0-63 and rows 64-127 run concurrently.
        q2 = persist.tile([P, HW], BF16)
        pixT_src = persist.tile([P, NB, C], BF16)
        pixT_ref = persist.tile([P, NB, C], BF16)
        wT_sb = persist.tile([P, 2, CQ], BF16)
        # Z row-sums per scores-pair come from ONE DVE reduce over the fused
        # [128, 2, 512] exp tile (axis X keeps the two j-blocks separate).
        # NOTE: do NOT try to split this as "ACT-accum mixed sum minus one
        # DVE half" -- Z magnitudes span e^+-34 across rows, so Z_jb1 =
        # M - Z_jb0 cancels catastrophically whenever Z_jb0 >> Z_jb1.
        zpart = persist.tile([P, NB // 2, 2, NS], F32)
        z_all = persist.tile([P, NB // 2, 2], F32)
        invz = persist.tile([P, NB // 2, 2], F32)
        onem = persist.tile([P, HW], BF16)       # (1 - mask) broadcast
        m_rep = persist.tile([P, HW], BF16)      # mask broadcast
        refb = persist.tile([P, 2, HW], BF16)    # ref stays resident (blend)
        tmpf = persist.tile([P, HW], BF16)       # finalize scratch
        izb = persist.tile([P, HW], BF16)        # 1/Z broadcast
        o_sb = persist.tile([P, 4, HW], BF16)
        tmp7 = persist.tile([P, SLICE], BF16)
        exp_bias = persist.tile([P, 1], F32)
        ident = persist.tile([P, P], F32)
        identb = persist.tile([P, P], BF16)      # moving operand for PE transposes
        invz_T = persist.tile([NB, P], F32)
        ones_st = persist.tile([1, P], BF16)     # stationary ones row (K=1)
        warm_sb = persist.tile([P, SLICE], BF16) # zeroed filler operand
        mask_sb = persist.tile([1, HW], BF16)    # mask as a single row
        zrowb = persist.tile([1, HW], BF16)      # 1/Z as a single row
        nc.vector.memset(exp_bias, -64.0)
        nc.vector.memset(ones_st, 1.0)
        make_identity(nc, ident)
        make_identity(nc, identb)


        with tc.tile_pool(name="early", bufs=1) as early:
            srcb = early.tile([P, 2, HW], BF16)

            # PE warmup: back-to-back matmuls on zeroed data keep the HAM
            # clock gate at 8/8 while input DMAs stream in.  warm_fill is
            # re-used at every spot where the PE would otherwise idle for
            # >1us waiting on a DMA -- a short utilization dip makes the
            # HAM drop to 4/8 and the re-grant takes ~85us.
            # non-trivial warm data: zero operands toggle no bits in the
            # PE, which the HAM's power estimate can read as idle
            nc.gpsimd.iota(
                warm_sb, [[1, SLICE]], base=1, channel_multiplier=3,
                allow_small_or_imprecise_dtypes=True,
            )

            def warm_fill(n):
                wp = ps_o.tile([P, SLICE], F32, name="warm", tag="pso")
                for _ in range(n):
                    nc.tensor.matmul(
                        wp, warm_sb[:, 0:P], warm_sb, start=True, stop=True
                    )

            warm_fill(16)

            # DMA order matters twice over: (a) cast lines must stay at
            # 8KB (16KB corrupts), (b) the DMA subsystem keeps only ~4
            # outstanding DMA instructions, recycling slots by completion
            # -- so the bulk input casts go back-to-back with nothing
            # data-gated between them, wT (needed by conv) leads, and the
            # mask row trails.  pixT is produced on the PE below, not by
            # XBAR transposes, so nothing else competes for the fabric.
            nc.sync.dma_start(out=wT_sb, in_=wT_r)
            JH = HW // 2
            for h in range(2):
                jh = slice(h * JH, (h + 1) * JH)
                for ci in range(2):
                    nc.gpsimd.dma_start(out=srcb[:, ci, jh], in_=src_r[:, ci, jh])
            for h in range(2):
                jh = slice(h * JH, (h + 1) * JH)
                for ci in range(2):
                    nc.gpsimd.dma_start(out=refb[:, ci, jh], in_=ref_r[:, ci, jh])
            for h in range(2):
                jh = slice(h * JH, (h + 1) * JH)
                nc.gpsimd.dma_start(
                    out=mask_sb[:, jh],
                    in_=mask.ap()[jh].partition_broadcast(1),
                )

            # 1x1 conv: q = wT.T @ src_pix; q into both partition halves
            for s in range(NS):
                if s == 4:
                    warm_fill(14)
                sl = ts(s, SLICE)
                psq = ps_s.tile([CQ, SLICE], F32, name="psq", tag="pss")
                for ci in range(2):
                    nc.tensor.matmul(
                        psq,
                        wT_sb[:, ci, :],
                        srcb[:, ci, sl],
                        start=(ci == 0),
                        stop=(ci == 1),
                    )
                nc.vector.tensor_copy(out=q2[0:CQ, sl], in_=psq)
                nc.vector.tensor_copy(out=q2[CQ:P, sl], in_=psq)
            warm_fill(4)

            # pixT[j, c] = pix[c, j] via matmul(lhsT=pix-block, rhs=I):
            # out[j, c'] = sum_c pix[c, j] I[c, c'].  4 j-blocks share
            # one PSUM bank; one DVE copy drains all 4.
            for ci in range(2):
                cs = slice(ci * P, (ci + 1) * P)
                for g in range(NB // 4):
                    psT = ps_s.tile([P, 4, P], F32, name="psT", tag="pss")
                    for q in range(4):
                        jb = g * 4 + q
                        nc.tensor.matmul(
                            psT[:, q, :], srcb[:, ci, ts(jb, P)],
                            identb, start=True, stop=True,
                        )
                    nc.vector.tensor_copy(
                        out=pixT_src[:, g * 4 : g * 4 + 4, cs], in_=psT
                    )

        def scores_pair(s, f_sb, jp):
            # pair psum tile spans 2 banks -> ONE [128,1024] ACTIVATE drains
            # both j-blocks ((N+352)/1.2ns: 1147 vs 2x720) and the Z row-sums
            # move to the idle DVE (per-pair reduce) instead of the ACT
            # accumulator reads (294ns each) -- the ACT stream stops gating
            # the PE's scores pairs.
            sl = ts(s, SLICE)
            jb0, jb1 = 2 * jp, 2 * jp + 1
            pss = ps_s.tile([P, 2, SLICE], F32, name="pss", tag="pss")
            nc.tensor.matmul(
                pss[:, 0, :], q2[0:CQ, ts(jb0, P)], q2[0:CQ, sl],
                start=True, stop=True, tile_position=(0, 0),
            )
            nc.tensor.matmul(
                pss[:, 1, :], q2[CQ:P, ts(jb1, P)], q2[CQ:P, sl],
                start=True, stop=True, tile_position=(CQ, 0),
            )
            nc.scalar.activation(
                out=f_sb[:, jb0 : jb0 + 2, :], in_=pss, func=EXP,
                bias=exp_bias,
            )
            nc.vector.reduce_sum(
                out=zpart[:, jp, :, s : s + 1],
                in_=f_sb[:, jb0 : jb0 + 2, :],
                axis=AX_X,
            )

        def scores_and_exp(s, f_sb):
            for jp in range(NB // 2):
                scores_pair(s, f_sb, jp)

        def apply_mm(s, f_sb, mid_hook=None, next_scores=None):
            # jb-major; the NEXT slice's scores pairs interleave into the
            # stream one pair per two j-blocks (8 matmuls ~ 1.8us), which
            # matches the ACT exp drain rate -- the scores phase then
            # costs no standalone PE time and the 4-deep pss pool never
            # blocks.  jb-major keeps the f-ring WAR in lockstep: the
            # pair for j-blocks (2k, 2k+1) lands right after this slice's
            # apply has consumed those same blocks.
            psos = [
                ps_o.tile([P, SLICE], F32, name=f"pso{cb}", tag="pso")
                for cb in range(4)
            ]
            for jb in range(NB):
                if jb == 10 and mid_hook is not None:
                    mid_hook()
                if next_scores is not None and jb % 2 == 0 and jb >= 2:
                    scores_pair(*next_scores, jb // 2 - 1)
                for cb in range(4):
                    pt = pixT_src if cb < 2 else pixT_ref
                    lhs = pt[:, jb, (cb % 2) * P : (cb % 2 + 1) * P]
                    nc.tensor.matmul(
                        psos[cb], lhs, f_sb[:, jb, :],
                        start=(jb == 0), stop=(jb == NB - 1),
                    )
            if next_scores is not None:
                scores_pair(*next_scores, NB // 2 - 1)
            return psos

        def copy_out(s, psos):
            # src_att: plain PSUM->SBUF copy; ref_att: fold (1-m) in
            sl = ts(s, SLICE)
            nc.vector.tensor_copy(out=o_sb[:, 0, sl], in_=psos[0])
            nc.vector.tensor_copy(out=o_sb[:, 1, sl], in_=psos[1])
            nc.vector.tensor_mul(o_sb[:, 2, sl], psos[2], onem[:, sl])
            nc.vector.tensor_mul(o_sb[:, 3, sl], psos[3], onem[:, sl])

        def izb_broadcast():
            # 1/Z row -> [128, HW] via ones-vector matmuls; ACT drains PSUM
            # (the DVE is busy with finalize at this point)
            for s2 in range(NS):
                sl2 = ts(s2, SLICE)
                psz = ps_s.tile([P, SLICE], F32, name="psz", tag="pss")
                nc.tensor.matmul(
                    psz, ones_st, zrowb[:, sl2], start=True, stop=True
                )
                nc.scalar.activation(out=izb[:, sl2], in_=psz, func=COPY)

        def finalize(lo, hi, dma_engines):
            """Normalize + blend + store for pixel columns [lo:hi)."""
            r = slice(lo, hi)
            for ci in range(2):
                nc.vector.tensor_mul(o_sb[:, 2 + ci, r], o_sb[:, 2 + ci, r], izb[:, r])
                nc.vector.tensor_mul(tmpf[:, r], m_rep[:, r], refb[:, ci, r])
                nc.vector.tensor_add(o_sb[:, 2 + ci, r], o_sb[:, 2 + ci, r], tmpf[:, r])
                nc.vector.tensor_mul(o_sb[:, ci, r], o_sb[:, ci, r], izb[:, r])
            # out rows: [flow(=cb2,3), src_att(=cb0,1)]
            for k, cb in enumerate([2, 3, 0, 1]):
                eng = dma_engines[k % len(dma_engines)]
                eng.dma_start(out=out_r[k, :, r], in_=o_sb[:, cb, r])

        with tc.tile_pool(name="fbuf", bufs=2) as fbuf:
            # double-buffered F ring: exp(s+1) writes one buffer while
            # apply(s) streams the other, so scores stay one slice ahead.
            # The ref-side PE transposes and the mask broadcast matmuls
            # slot in after scores(0)/scores(1) -- by then refb / mask_sb
            # have landed, so the PE never stalls on them.
            fbufs = [
                fbuf.tile([P, NB, SLICE], BF16, name="f_sb", tag="f")
                for _ in range(2)
            ]
            # scores(0) and scores(1) pairs are ACT-drain-paced (1.4us
            # per pair vs 0.46us of PE work); the ref-side PE transposes
            # interleave into the stall slots -- one 4-block group per
            # two pairs -- so the PE stays dense and no standalone T_ref
            # phase remains.
            def tref_group(ci, g):
                cs = slice(ci * P, (ci + 1) * P)
                psT = ps_s.tile([P, 4, P], F32, name="psT", tag="pss")
                for q in range(4):
                    jb = g * 4 + q
                    nc.tensor.matmul(
                        psT[:, q, :], refb[:, ci, ts(jb, P)],
                        identb, start=True, stop=True,
                    )
                nc.vector.tensor_copy(
                    out=pixT_ref[:, g * 4 : g * 4 + 4, cs], in_=psT
                )

            tgroups = [(ci, g) for ci in range(2) for g in range(NB // 4)]
            ti = 0
            for k in range(2 * (NB // 2)):
                s01, jp = divmod(k, NB // 2)
                scores_pair(s01, fbufs[s01], jp)
                if k % 2 == 1 and ti < len(tgroups):
                    tref_group(*tgroups[ti])
                    ti += 1
            while ti < len(tgroups):
                tref_group(*tgroups[ti])
                ti += 1
            # mask column-broadcast + drains (DVE: m and 1-m)
            for s in range(NS):
                sl = ts(s, SLICE)
                psm = ps_s.tile([P, SLICE], F32, name="psm", tag="pss")
                nc.tensor.matmul(
                    psm, ones_st, mask_sb[:, sl], start=True, stop=True
                )
                nc.vector.tensor_copy(out=m_rep[:, sl], in_=psm)
                nc.vector.tensor_scalar(
                    out=onem[:, sl], in0=psm, scalar1=-1.0, scalar2=1.0,
                    op0=mybir.AluOpType.mult, op1=mybir.AluOpType.add,
                )
            for s in range(NS - 2):
                psos = apply_mm(
                    s, fbufs[s % 2], next_scores=(s + 2, fbufs[s % 2])
                )
                copy_out(s, psos)
            s7 = NS - 1
            f_cur = fbufs[(NS - 2) % 2]
            f_sb = fbufs[s7 % 2]
            # Z partials all land once exp(s7) retires (during apply(s6));
            # issuing the DVE reduce before copy_out(s6) lets it run early
            nc.vector.reduce_sum(out=z_all, in_=zpart, axis=AX_X)
            nc.vector.reciprocal(out=invz, in_=z_all)
            psos = apply_mm(s7 - 1, f_cur)
            copy_out(s7 - 1, psos)
            ps_t = ps_s.tile([NB, P], F32, name="ps_t", tag="pss")
            nc.tensor.transpose(
                ps_t, invz.rearrange("p a b -> p (a b)"), ident
            )
            nc.vector.tensor_copy(out=invz_T, in_=ps_t)
            # flatten [32 partitions, 128] -> one [1, 4096] row (SBUF->SBUF
            # DMA crosses partitions; f32 -> bf16 converts on the way)
            nc.gpsimd.dma_start(
                out=zrowb.rearrange("a (b q) -> a b q", q=P), in_=invz_T
            )
            # slice 7 apply; after 10 j-blocks the zrowb row has landed, so
            # the izb broadcast matmuls slot into the middle of the stream
            psos7 = apply_mm(s7, f_sb, mid_hook=izb_broadcast)
            finalize(0, (NS - 1) * SLICE, [nc.sync, nc.scalar, nc.gpsimd])
            # slice 7: copy-out doubles as normalize + blend
            sl7 = ts(s7, SLICE)
            nc.vector.tensor_mul(tmp7, onem[:, sl7], izb[:, sl7])
            nc.vector.tensor_mul(o_sb[:, 0, sl7], psos7[0], izb[:, sl7])
            nc.vector.tensor_mul(o_sb[:, 1, sl7], psos7[1], izb[:, sl7])
            for ci in range(2):
                nc.vector.tensor_mul(o_sb[:, 2 + ci, sl7], psos7[2 + ci], tmp7)
                nc.vector.tensor_mul(tmpf[:, sl7], m_rep[:, sl7], refb[:, ci, sl7])
                nc.vector.tensor_add(
                    o_sb[:, 2 + ci, sl7], o_sb[:, 2 + ci, sl7], tmpf[:, sl7]
                )
            for k, cb in enumerate([2, 3, 0, 1]):
                eng = [nc.sync, nc.scalar, nc.gpsimd, nc.sync][k]
                eng.dma_start(out=out_r[k, :, sl7], in_=o_sb[:, cb, sl7])

            if dbg is not None:
                nc.sync.dma_start(out=dbg["q2"].ap(), in_=q2)
                nc.sync.dma_start(
                    out=dbg["zpart"].ap().rearrange("p (b s) -> p b s", s=NS),
                    in_=zpart,
                )
                nc.sync.dma_start(out=dbg["invz"].ap(), in_=invz)
                nc.sync.dma_start(out=dbg["izb"].ap(), in_=izb)
                nc.sync.dma_start(out=dbg["onem"].ap(), in_=onem)

                nc.sync.dma_start(
                    out=dbg["f7"].ap().rearrange("p (b i) -> p b i", b=NB),
                    in_=f_sb,
                )


def build():
    nc = bacc.Bacc(
        "TRN2",
        target_bir_lowering=False,
        debug=False,
        enable_asserts=False,
        num_devices=NCORES,
    )
    src = nc.dram_tensor("src", (C, HW), F32, kind="ExternalInput")
    ref = nc.dram_tensor("ref", (C, HW), F32, kind="ExternalInput")
    mask = nc.dram_tensor("mask", (HW,), F32, kind="ExternalInput")
    wT = nc.dram_tensor("wT", (C, CQ), BF16, kind="ExternalInput")
    out = nc.dram_tensor("out", (2 * C, HW), BF16, kind="ExternalOutput")
    with tile.TileContext(nc) as tc:
        _build_body(tc, src, ref, mask, wT, out)
    nc.compile()
    return nc


_CACHE = {}


def _get_nc():
    if "nc" not in _CACHE:
        _CACHE["nc"] = build()
    return _CACHE["nc"]


def _in_maps(src_mask, src_feature, ref_feature, conv_w):
    import ml_dtypes

    n_batch = src_feature.shape[0]
    wT = np.ascontiguousarray(
        np.asarray(conv_w, dtype=np.float32).T.astype(ml_dtypes.bfloat16)
    )
    maps = []
    for n in range(n_batch):
        maps.append(
            {
                "src": np.ascontiguousarray(
                    np.asarray(src_feature[n], dtype=np.float32).reshape(C, HW)
                ),
                "ref": np.ascontiguousarray(
                    np.asarray(ref_feature[n], dtype=np.float32).reshape(C, HW)
                ),
                "mask": np.ascontiguousarray(
                    np.asarray(src_mask[n], dtype=np.float32).reshape(HW)
                ),
                "wT": wT,
            }
        )
    return maps


def _install_ntff_hook():
    """The agent image's antenv lacks axon_hooks; recreate it so
    run_bass_kernel_spmd(trace=True) can capture NTFF profiles."""
    import sys
    import types

    if "antenv.axon_hooks" in sys.modules:
        return
    import antenv
    from trn_agent_boot.trn_boot import _ntff_profile_via_ctypes

    hook = _ntff_profile_via_ctypes("/opt/axon/libaxon_pjrt.so")
    mod = types.ModuleType("antenv.axon_hooks")
    mod._hook = hook
    mod.set_axon_ntff_profile_hook = lambda h: setattr(mod, "_hook", h)
    mod.get_axon_ntff_profile_hook = lambda: mod._hook
    sys.modules["antenv.axon_hooks"] = mod
    antenv.axon_hooks = mod


def run(src_mask, src_feature, ref_feature, conv_w, trace=False):
    """Run on 8 NeuronCores. Returns (output [N,2C,H,W], BassKernelResults)."""
    n_batch, c, h, w = src_feature.shape
    if trace:
        _install_ntff_hook()
    nc = _get_nc()
    maps = _in_maps(src_mask, src_feature, ref_feature, conv_w)
    res = bass_utils.run_bass_kernel_spmd(
        nc, maps, core_ids=list(range(NCORES)), trace=trace
    )
    out = np.stack([np.asarray(r["out"]) for r in res.results], axis=0)
    return out.reshape(n_batch, 2 * c, h, w).astype(np.float32), res


def kernel(src_mask, src_feature, ref_feature, conv_w):
    out, _ = run(src_mask, src_feature, ref_feature, conv_w)
    return out



# revision 75
# speedup vs baseline: 1.1390x; 1.0521x over previous
"""Trainium2 Bass kernel for ExampleGuidedAttention (N=8, C=256, H=W=64).

Data-parallel over batch N across 8 NeuronCores; each core computes one
batch element's full guided attention.

Algorithm notes (per core):
  q = conv_w @ src_pix                      [64, 4096]   (PE, bf16)
  S^T[j,i] = sum_o q[o,j] q[o,i]            (PE, bf16; S symmetric; two
             j-blocks packed in the 128x128 array via tile_position
             (row groups 0-63 / 64-127) since the contraction is only 64)
  F[j,i] = exp(S^T[j,i] - 64)               (ACT; global shift keeps fp32
             exp in range -- softmax ratio unchanged; diag scores are
             chi2(64) so they reach ~120).  The ACT accumulator
             (accum_out) yields Z partials for free: Z[j] = sum_i F[j,i]
             equals the softmax denominator because S is symmetric.
  O[c,i] = sum_j pixT[j,c] * F[j,i]         (PE, bf16, natural layout)
  out    = [ (1-m)*ref_att*invZ + m*ref ; src_att*invZ ]

Performance structure (vs the v1 kernel):
  - inputs land as 8KB-contiguous partition lines ([128, 2048] convert
    chunks): ~2x the bandwidth of the 2KB-line version.  NOTE: 16KB
    lines silently corrupt (descriptor length-field limit) -- keep
    convert-DMA lines at <= 8KB.
  - all [128, HW] column-broadcasts (mask, 1/Z) are ones-vector matmuls
    on the PE into PSUM -- the partition_broadcast DMA path is
    software-dynamic and costs ~25us per 2MB.
  - Z comes free from the exp pass via the ACT accumulator.
  - finalize is restructured as  flow = [(1-m)*ra]*izb + m*ref  with
    (1-m) folded into the PSUM->SBUF copy-out of each slice and m*ref
    precomputed during the input phase, so the tail after the last
    apply matmul is only slice 7's small DVE ops + 4 small DMAs.
  - scores for slice s+1 are issued before apply(s) on a double-buffered
    F ring: the PE never waits on the ACT exp stream.
  - everything downstream of PSUM is bf16 (output DRAM tensor too;
    host casts back to f32) -- halves DVE and output-DMA cost.
"""

import numpy as np

import concourse.bass as bass
import concourse.mybir as mybir
import concourse.tile as tile
from concourse import bacc, bass_utils
from concourse.bass import ts
from concourse.masks import make_identity

P = 128
C = 256          # feature channels
CQ = 64          # query channels
HW = 4096        # pixels per image
NB = HW // P     # 32 pixel blocks (contraction chunks)
SLICE = 512
NS = HW // SLICE  # 8 output column slices
NCORES = 8

F32 = mybir.dt.float32
BF16 = mybir.dt.bfloat16
EXP = mybir.ActivationFunctionType.Exp
COPY = mybir.ActivationFunctionType.Copy
AX_X = mybir.AxisListType.X


def _build_body(tc, src, ref, mask, wT, out, dbg=None):
    nc = tc.nc
    src_r = src.ap().rearrange("(ci p) j -> p ci j", p=P)   # [128, 2, 4096]
    ref_r = ref.ap().rearrange("(ci p) j -> p ci j", p=P)
    wT_r = wT.ap().rearrange("(ci p) o -> p ci o", p=P)     # [128, 2, 64]
    out_r = out.ap().rearrange("(cb p) j -> cb p j", p=P)   # [4, 128, 4096]

    with (
        tc.tile_pool(name="persist", bufs=1) as persist,
        tc.tile_pool(name="ps_s", bufs=2, space="PSUM") as ps_s,
        tc.tile_pool(name="ps_o", bufs=4, space="PSUM") as ps_o,
    ):
        # q duplicated into both partition halves so scores matmuls can be
        # row-packed: tile at rows 0-63 and rows 64-127 run concurrently.
        q2 = persist.tile([P, HW], BF16)
        pixT_src = persist.tile([P, NB, C], BF16)
        pixT_ref = persist.tile([P, NB, C], BF16)
        wT_sb = persist.tile([P, 2, CQ], BF16)
        # Z row-sums per scores-pair come from ONE DVE reduce over the fused
        # [128, 2, 512] exp tile (axis X keeps the two j-blocks separate).
        # NOTE: do NOT try to split this as "ACT-accum mixed sum minus one
        # DVE half" -- Z magnitudes span e^+-34 across rows, so Z_jb1 =
        # M - Z_jb0 cancels catastrophically whenever Z_jb0 >> Z_jb1.
        zpart = persist.tile([P, NB // 2, 2, NS], F32)
        z_all = persist.tile([P, NB // 2, 2], F32)
        invz = persist.tile([P, NB // 2, 2], F32)
        onem = persist.tile([P, HW], BF16)       # (1 - mask) broadcast
        # refb holds ref during the tref transposes, then is overwritten in
        # place with m*ref (the blend addend) by the mask-broadcast drains
        # -- no separate m_rep / tmpf tiles (SBUF).
        refb = persist.tile([P, 2, HW], BF16)
        izb = persist.tile([P, HW], BF16)        # 1/Z broadcast
        o_sb = persist.tile([P, 4, HW], BF16)
        # srcb lives in the persist pool: if it were freed before the f ring
        # allocates, the allocator recycles its SBUF region and exp(0)
        # inherits a false WAR on every srcb reader (conv + transposes),
        # which pushed the first ACTIVATE out to ~40us.
        srcb = persist.tile([P, 2, HW], BF16)
        exp_bias = persist.tile([P, 1], F32)
        ident = persist.tile([P, P], F32)
        identb = persist.tile([P, P], BF16)      # moving operand for PE transposes
        invz_T = persist.tile([NB, P], F32)
        ones_st = persist.tile([1, P], BF16)     # stationary ones row (K=1)
        warm_sb = persist.tile([P, SLICE], BF16) # zeroed filler operand
        mask_sb = persist.tile([1, HW], BF16)    # mask as a single row
        zrowb = persist.tile([1, HW], BF16)      # 1/Z as a single row
        act_warm = persist.tile([P, 1], F32)
        nc.vector.memset(exp_bias, -64.0)
        nc.vector.memset(ones_st, 1.0)
        make_identity(nc, ident)
        make_identity(nc, identb)
        # RANDOM warm data (max bit-toggle): constant-value warm operands
        # defeat the PE power estimator -- HAM reports 8/8 yet the whole
        # run executes at ~1.2GHz (+60us).  DVE random avoids the gpsimd
        # iota that would delay the input-DMA queue.
        nc.vector.random(warm_sb)
        # dummy exp: walrus inserts the ACT_TABLE_LOAD (~2.7us) before the
        # first Exp -- pay it here, under the input DMAs, not inside exp(0)
        nc.scalar.activation(out=act_warm, in_=exp_bias, func=EXP)


        if True:
            # PE warmup: back-to-back matmuls on constant nonzero data keep
            # the PE continuously busy until the first src chunk lands
            # (~13.5us) -- the HAM un-throttles to 8/8 after ~3.4us of
            # sustained activity, so conv/transposes/scores then run at
            # 2.4GHz instead of the cold 1.2GHz.
            def warm_fill(n):
                wp = ps_o.tile([P, SLICE], F32, name="warm", tag="pso")
                for _ in range(n):
                    nc.tensor.matmul(
                        wp, warm_sb[:, 0:P], warm_sb, start=True, stop=True
                    )

            # 44 matmuls span ~11us even if the first ~8 run cold (427ns at
            # 1.2GHz before the HAM lifts at ~3.4us) -- bridging the idle
            # window between warmup and the first src chunk (~14us).  A
            # shorter fill leaves a >2us idle there, and when the HAM's
            # free-running MID window catches it, the PE re-throttles to
            # 4/8 for ~34us right across conv/scores/apply(0) (+10us).
            warm_fill(32)

            # Input DMAs: ONE queue, priority order.  The DMA subsystem fans
            # one queue's descriptors across all 16 engines (~225GB/s
            # aggregate cap per core); multiple queues only make src, mask
            # and ref COMPETE so everything lands late together.  bf16
            # host-cast inputs halve the bytes vs the old f32 cast-DMAs:
            # src done ~11us, mask ~12, ref ~20.
            nc.sync.dma_start(out=wT_sb, in_=wT_r)
            JH = HW // 2
            for h in range(2):
                jh = slice(h * JH, (h + 1) * JH)
                for ci in range(2):
                    nc.gpsimd.dma_start(
                        out=srcb[:, ci, jh], in_=src_r[:, ci, jh]
                    )
            for h in range(2):
                jh = slice(h * JH, (h + 1) * JH)
                nc.gpsimd.dma_start(
                    out=mask_sb[:, jh],
                    in_=mask.ap()[jh].partition_broadcast(1),
                )
            for h in range(2):
                jh = slice(h * JH, (h + 1) * JH)
                for ci in range(2):
                    nc.gpsimd.dma_start(
                        out=refb[:, ci, jh], in_=ref_r[:, ci, jh]
                    )

            # pixT[j, c] = pix[c, j] via matmul(lhsT=pix-block, rhs=I):
            # out[j, c'] = sum_c pix[c, j] I[c, c'].  4 j-blocks share one
            # PSUM bank; one copy drains all 4.  The tiles live in the ps_o
            # ring (idle until apply(0)) so the conv/transpose chain never
            # shares the 2-deep pss ring with the ACT-paced scores pairs.
            # drain engine: DVE for the head (ACT is busy with exp(0)),
            # ACT for the tref groups inside apply(0) (DVE does Z there).
            def t_group(pix_in, pix_out, ci, g, drain):
                # head (drain=dve): ps_o ring -- free until apply(0), keeps
                # the conv/T chain off the scores ring.  inside apply(0)
                # (drain=act): ps_s ring -- ps_o's 4 slots are the apply
                # accumulators then.
                cs = slice(ci * P, (ci + 1) * P)
                if drain == "act":
                    psT = ps_s.tile([P, 4, P], F32, name="psT", tag="pss")
                else:
                    psT = ps_o.tile([P, 4, P], F32, name="psT", tag="pso")
                for q in range(4):
                    jb = g * 4 + q
                    nc.tensor.matmul(
                        psT[:, q, :], pix_in[:, ci, ts(jb, P)],
                        identb, start=True, stop=True,
                    )
                if drain == "act":
                    nc.scalar.activation(
                        out=pix_out[:, g * 4 : g * 4 + 4, cs], in_=psT,
                        func=COPY,
                    )
                else:
                    nc.vector.tensor_copy(
                        out=pix_out[:, g * 4 : g * 4 + 4, cs], in_=psT
                    )

        def scores_pair(s, f_sb, jp, fused=None):
            # Hybrid Z so neither ACT nor DVE saturates: even jp -> ONE
            # fused [128,1024] ACTIVATE (1147ns vs 2x720), Z row-sums later
            # via a DVE z_reduce; odd jp -> two per-jb ACTIVATEs whose ACT
            # accumulator yields clean per-jb Z for free (+294ns reads).
            # Per slice: ACT ~25us, DVE ~12us -- both under the ~34.5us
            # apply window.  (A fused ACTIVATE's accumulator would MIX the
            # two j-blocks' row sums, and un-mixing cancels catastrophically
            # since Z spans e^+-34 -- hence the per-jb unfused form.)
            sl = ts(s, SLICE)
            jb0, jb1 = 2 * jp, 2 * jp + 1
            pss = ps_s.tile([P, 2, SLICE], F32, name="pss", tag="pss")
            nc.tensor.matmul(
                pss[:, 0, :], q2[0:CQ, ts(jb0, P)], q2[0:CQ, sl],
                start=True, stop=True, tile_position=(0, 0),
            )
            nc.tensor.matmul(
                pss[:, 1, :], q2[CQ:P, ts(jb1, P)], q2[CQ:P, sl],
                start=True, stop=True, tile_position=(CQ, 0),
            )
            if fused is None:
                fused = jp % 2 == 0
            if fused:
                nc.scalar.activation(
                    out=f_sb[:, jb0 : jb0 + 2, :], in_=pss, func=EXP,
                    bias=exp_bias,
                )
            else:
                for h in range(2):
                    nc.scalar.activation(
                        out=f_sb[:, jb0 + h, :], in_=pss[:, h, :], func=EXP,
                        bias=exp_bias,
                        accum_out=zpart[:, jp, h, s : s + 1],
                    )

        def z_reduce(f_sb, s, jp):
            # Z row-sums for a FUSED pair (even jp) -- DVE, axis X keeps
            # the two j-blocks separate
            nc.vector.reduce_sum(
                out=zpart[:, jp, :, s : s + 1],
                in_=f_sb[:, 2 * jp : 2 * jp + 2, :],
                axis=AX_X,
            )

        def apply_mm(s, f_sb, mid_hook=None, next_scores=None, odd_hook=None,
                     zred=True):
            # jb-major; the NEXT slice's scores pairs interleave into the
            # stream one pair per two j-blocks (8 matmuls ~ 1.8us), which
            # matches the ACT exp drain rate -- the scores phase then
            # costs no standalone PE time and the pss pool never blocks.
            # jb-major keeps the f-ring WAR in lockstep: the pair for
            # j-blocks (2k, 2k+1) lands right after this slice's apply has
            # consumed those same blocks.  odd_hook(jb) lets extra work
            # (tref groups, a second scores set) slot into odd j-blocks.
            psos = [
                ps_o.tile([P, SLICE], F32, name=f"pso{cb}", tag="pso")
                for cb in range(4)
            ]
            for jb in range(NB):
                if jb == 12 and mid_hook is not None:
                    mid_hook()
                if next_scores is not None and jb % 2 == 0 and jb >= 2:
                    scores_pair(*next_scores, jb // 2 - 1)
                if zred and jb % 4 == 0:
                    # this slice's fused-pair Z reduces: pair jb//2 is read
                    # here just before exp(s+2) overwrites it at jb+2
                    z_reduce(f_sb, s, jb // 2)
                if odd_hook is not None and jb % 2 == 1:
                    odd_hook(jb)
                for cb in range(4):
                    pt = pixT_src if cb < 2 else pixT_ref
                    lhs = pt[:, jb, (cb % 2) * P : (cb % 2 + 1) * P]
                    nc.tensor.matmul(
                        psos[cb], lhs, f_sb[:, jb, :],
                        start=(jb == 0), stop=(jb == NB - 1),
                    )
            if next_scores is not None:
                scores_pair(*next_scores, NB // 2 - 1)
            return psos

        def copy_out(s, psos):
            # plain PSUM->SBUF copies; all mask/1/Z algebra happens in
            # finalize as  flow = ref + (1-m)*(ra*izb - ref)
            sl = ts(s, SLICE)
            for cb in range(4):
                nc.vector.tensor_copy(out=o_sb[:, cb, sl], in_=psos[cb])

        def izb_broadcast():
            # 1/Z row -> [128, HW] via ones-vector matmuls; ACT drains PSUM
            # (the DVE is busy with finalize at this point)
            for s2 in range(NS):
                sl2 = ts(s2, SLICE)
                psz = ps_s.tile([P, SLICE], F32, name="psz", tag="pss")
                nc.tensor.matmul(
                    psz, ones_st, zrowb[:, sl2], start=True, stop=True
                )
                nc.scalar.activation(out=izb[:, sl2], in_=psz, func=COPY)

        def finalize(lo, hi, dma_engines):
            """Normalize + blend + store for pixel columns [lo:hi).

            flow = (1-m)*ra*izb + m*ref  ==  ref + (1-m)*(ra*izb - ref),
            which needs only onem and RAW refb (no m*ref precompute)."""
            r = slice(lo, hi)
            for ci in range(2):
                o2 = o_sb[:, 2 + ci, r]
                nc.vector.tensor_mul(o2, o2, izb[:, r])
                nc.vector.tensor_sub(o2, o2, refb[:, ci, r])
                nc.vector.tensor_mul(o2, o2, onem[:, r])
                nc.vector.tensor_add(o2, o2, refb[:, ci, r])
                nc.vector.tensor_mul(o_sb[:, ci, r], o_sb[:, ci, r], izb[:, r])
            # out rows: [flow(=cb2,3), src_att(=cb0,1)]
            for k, cb in enumerate([2, 3, 0, 1]):
                eng = dma_engines[k % len(dma_engines)]
                eng.dma_start(out=out_r[k, :, r], in_=o_sb[:, cb, r])

        with tc.tile_pool(name="fbuf", bufs=2) as fbuf:
            # double-buffered F ring: exp(s+1) writes one buffer while
            # apply(s) streams the other.
            fbufs = [
                fbuf.tile([P, NB, SLICE], BF16, name="f_sb", tag="f")
                for _ in range(2)
            ]
            # conv + src transposes + scores(0), all paced by the src DMA:
            # conv slice s frees exactly the q columns that scores pairs
            # 2s/2s+1 need, so exp(0) STREAMS BEHIND THE SRC DMA and f(0)
            # is complete ~2us after src lands (vs +18us if scores waited
            # for the whole conv+transpose phase in the in-order PE queue).
            for s in range(NS):
                sl = ts(s, SLICE)
                psq = ps_o.tile([CQ, SLICE], F32, name="psq", tag="pso")
                for ci in range(2):
                    nc.tensor.matmul(
                        psq,
                        wT_sb[:, ci, :],
                        srcb[:, ci, sl],
                        start=(ci == 0),
                        stop=(ci == 1),
                    )
                nc.vector.tensor_copy(out=q2[0:CQ, sl], in_=psq)
                nc.vector.tensor_copy(out=q2[CQ:P, sl], in_=psq)
                t_group(srcb, pixT_src, 0, s, "dve")
                t_group(srcb, pixT_src, 1, s, "dve")
                # slice-0 exps ALL-FUSED: the hybrid's 25.4us of ACT per
                # slice would out-pace the src DMA here and delay apply(0);
                # fused is 18.4us.  Slice-0 Z reduces all happen on the DVE
                # inside apply(0)'s two halves.
                scores_pair(0, fbufs[0], 2 * s, fused=True)
                scores_pair(0, fbufs[0], 2 * s + 1, fused=True)

            # apply(0) is split into halves so it can START as soon as src
            # is done (~26us): the SRC half (cb0/1) needs only pixT_src +
            # f(0) and carries scores(1) + the slice-0 Z reduces; the REF
            # half starts ~19us later, by which time the whole ref tensor
            # has landed, so the tref transposes (odd j-blocks, one group
            # ahead of use) never stall on the ref DMA.
            sl0 = ts(0, SLICE)
            psosA = [
                ps_o.tile([P, SLICE], F32, name=f"ps0s{cb}", tag="pso")
                for cb in range(2)
            ]
            for jb in range(NB):
                if jb % 4 == 2:
                    scores_pair(1, fbufs[1], (jb - 2) // 4)
                if jb % 4 == 0:
                    z_reduce(fbufs[0], 0, jb // 2)
                for cb in range(2):
                    lhs = pixT_src[:, jb, (cb % 2) * P : (cb % 2 + 1) * P]
                    nc.tensor.matmul(
                        psosA[cb], lhs, fbufs[0][:, jb, :],
                        start=(jb == 0), stop=(jb == NB - 1),
                    )
            nc.vector.tensor_copy(out=o_sb[:, 0, sl0], in_=psosA[0])
            nc.vector.tensor_copy(out=o_sb[:, 1, sl0], in_=psosA[1])

            t_group(refb, pixT_ref, 0, 0, "act")
            t_group(refb, pixT_ref, 1, 0, "act")
            psosB = [
                ps_o.tile([P, SLICE], F32, name=f"ps0r{cb}", tag="pso")
                for cb in range(2)
            ]
            for jb in range(NB):
                if jb % 4 == 2:
                    scores_pair(1, fbufs[1], (jb - 2) // 4 + 8)
                if jb % 4 == 0:
                    z_reduce(fbufs[0], 0, jb // 2 + 1)
                if jb % 4 == 1 and jb // 4 + 1 < NB // 4:
                    t_group(refb, pixT_ref, 0, jb // 4 + 1, "act")
                elif jb % 4 == 3 and (jb - 3) // 4 + 1 < NB // 4:
                    t_group(refb, pixT_ref, 1, (jb - 3) // 4 + 1, "act")
                for cb in range(2):
                    lhs = pixT_ref[:, jb, cb * P : (cb + 1) * P]
                    nc.tensor.matmul(
                        psosB[cb], lhs, fbufs[0][:, jb, :],
                        start=(jb == 0), stop=(jb == NB - 1),
                    )
            nc.vector.tensor_copy(out=o_sb[:, 2, sl0], in_=psosB[0])
            nc.vector.tensor_copy(out=o_sb[:, 3, sl0], in_=psosB[1])
            # apply(1) carries TWO scores sets (sc2 even, sc3 odd) so the
            # steady lookahead-2 cadence resumes at apply(2) and scores(7)
            # still retires during apply(5) (the invz chain needs that).
            # sc(3) rides apply(1)'s odd j-blocks ALL-FUSED: apply(1)
            # already hosts exp(2)'s hybrid 25.4us; a second hybrid set
            # would put 50.8us of ACT into a ~36us window.  Slice-3's odd
            # Z reduces run on the DVE during apply(2) (even ones via
            # apply(3)'s standard zred hook).
            def sc3_hook(jb):
                if jb >= 3:
                    scores_pair(3, fbufs[1], (jb - 3) // 2, fused=True)

            psos = apply_mm(
                1, fbufs[1], next_scores=(2, fbufs[0]), odd_hook=sc3_hook
            )
            scores_pair(3, fbufs[1], NB // 2 - 1, fused=True)
            copy_out(1, psos)
            # mask column-broadcast (onem = 1-m) rides apply(3)'s odd
            # j-blocks -- it is only needed by the finalize at apply(6)
            def mask_hook(jb):
                s2 = jb // 2
                if jb % 2 == 1 and s2 < NS:
                    sl = ts(s2, SLICE)
                    psm = ps_s.tile([P, SLICE], F32, name="psm", tag="pss")
                    nc.tensor.matmul(
                        psm, ones_st, mask_sb[:, sl], start=True, stop=True
                    )
                    nc.scalar.activation(
                        out=onem[:, sl], in_=psm, func=COPY, scale=-1.0,
                        bias=1.0,
                    )

            def r3_hook(jb):
                if jb % 4 == 1:
                    z_reduce(fbufs[1], 3, (jb - 1) // 2 + 1)

            for s in range(2, NS - 2):
                if s == 2:
                    oh = r3_hook
                elif s == 3:
                    oh = mask_hook
                else:
                    oh = None
                psos = apply_mm(
                    s, fbufs[s % 2], next_scores=(s + 2, fbufs[s % 2]),
                    odd_hook=oh,
                )
                copy_out(s, psos)
            s7 = NS - 1
            f_cur = fbufs[(NS - 2) % 2]
            f_sb = fbufs[s7 % 2]
            # slices 6/7 are never overwritten, so their fused-pair Z
            # reduces run here (executing during apply(5)'s tail / the
            # apply(6) ramp); the unfused halves came from the ACT
            # accumulator at exp time.  Then the whole invz -> transpose ->
            # zrowb chain runs before apply(s6), so izb can broadcast
            # DURING apply(s6) and the DVE finalize of slices 0-6 + their
            # output DMAs hide under apply(s7).
            for jp in range(0, NB // 2, 2):
                z_reduce(f_cur, NS - 2, jp)
                z_reduce(f_sb, s7, jp)
            nc.vector.reduce_sum(out=z_all, in_=zpart, axis=AX_X)
            nc.vector.reciprocal(out=invz, in_=z_all)
            ps_t = ps_s.tile([NB, P], F32, name="ps_t", tag="pss")
            nc.tensor.transpose(
                ps_t, invz.rearrange("p a b -> p (a b)"), ident
            )
            nc.vector.tensor_copy(out=invz_T, in_=ps_t)
            # flatten [32 partitions, 128] -> one [1, 4096] row (SBUF->SBUF
            # DMA crosses partitions; f32 -> bf16 converts on the way)
            nc.gpsimd.dma_start(
                out=zrowb.rearrange("a (b q) -> a b q", q=P), in_=invz_T
            )
            psos = apply_mm(s7 - 1, f_cur, mid_hook=izb_broadcast, zred=False)
            copy_out(s7 - 1, psos)
            # per-slice finalize: each slice's 4 output DMAs fire as soon
            # as ITS DVE ops are done, so the ~4MB output streams out
            # under apply(7) instead of queueing behind one big finalize
            # one-shot finalize over slices 0-6: the [128, 3584] output
            # DMAs keep 7KB partition lines -- per-slice 512-col chunks
            # (1KB lines) drop the output DMA to ~70GB/s and cost +60us
            finalize(0, (NS - 1) * SLICE, [nc.sync, nc.scalar, nc.gpsimd])
            # slice 7 runs CB-MAJOR: each output row's 32-j-block psum
            # group completes after ~7us, so its normalize + blend + store
            # overlaps the next row's matmul stream -- only the last row's
            # short DVE/DMA chain remains after the final matmul.
            sl7 = ts(s7, SLICE)
            engs7 = [nc.sync, nc.scalar, nc.gpsimd, nc.sync]
            for k, cb in enumerate([2, 3, 0, 1]):
                pso = ps_o.tile([P, SLICE], F32, name=f"ps7{cb}", tag="pso")
                pt = pixT_src if cb < 2 else pixT_ref
                for jb in range(NB):
                    lhs = pt[:, jb, (cb % 2) * P : (cb % 2 + 1) * P]
                    nc.tensor.matmul(
                        pso, lhs, f_sb[:, jb, :],
                        start=(jb == 0), stop=(jb == NB - 1),
                    )
                o2 = o_sb[:, cb, sl7]
                nc.vector.tensor_mul(o2, pso, izb[:, sl7])
                if cb >= 2:
                    nc.vector.tensor_sub(o2, o2, refb[:, cb - 2, sl7])
                    nc.vector.tensor_mul(o2, o2, onem[:, sl7])
                    nc.vector.tensor_add(o2, o2, refb[:, cb - 2, sl7])
                engs7[k].dma_start(out=out_r[k, :, sl7], in_=o2)

            if dbg is not None:
                nc.sync.dma_start(out=dbg["q2"].ap(), in_=q2)
                nc.sync.dma_start(
                    out=dbg["zpart"].ap().rearrange("p (b s) -> p b s", s=NS),
                    in_=zpart,
                )
                nc.sync.dma_start(out=dbg["invz"].ap(), in_=invz)
                nc.sync.dma_start(out=dbg["izb"].ap(), in_=izb)
                nc.sync.dma_start(out=dbg["onem"].ap(), in_=onem)

                nc.sync.dma_start(
                    out=dbg["f7"].ap().rearrange("p (b i) -> p b i", b=NB),
                    in_=f_sb,
                )


def build():
    nc = bacc.Bacc(
        "TRN2",
        target_bir_lowering=False,
        debug=False,
        enable_asserts=False,
        num_devices=NCORES,
    )
    # src/ref stay f32 + gpsimd cast-DMAs: measured ~235GB/s with 8KB read
    # lines, vs ~70GB/s for bf16 static loads (4KB lines) -- the cast path
    # is the FAST one here.  mask is host-cast bf16 (tiny).
    src = nc.dram_tensor("src", (C, HW), F32, kind="ExternalInput")
    ref = nc.dram_tensor("ref", (C, HW), F32, kind="ExternalInput")
    mask = nc.dram_tensor("mask", (HW,), BF16, kind="ExternalInput")
    wT = nc.dram_tensor("wT", (C, CQ), BF16, kind="ExternalInput")
    out = nc.dram_tensor("out", (2 * C, HW), BF16, kind="ExternalOutput")
    with tile.TileContext(nc) as tc:
        _build_body(tc, src, ref, mask, wT, out)
    nc.compile()
    return nc


_CACHE = {}


def _get_nc():
    if "nc" not in _CACHE:
        _CACHE["nc"] = build()
    return _CACHE["nc"]


def _in_maps(src_mask, src_feature, ref_feature, conv_w):
    import ml_dtypes

    n_batch = src_feature.shape[0]
    wT = np.ascontiguousarray(
        np.asarray(conv_w, dtype=np.float32).T.astype(ml_dtypes.bfloat16)
    )
    mask_bf = np.asarray(src_mask, dtype=np.float32).reshape(
        n_batch, HW
    ).astype(ml_dtypes.bfloat16)
    maps = []
    for n in range(n_batch):
        maps.append(
            {
                "src": np.ascontiguousarray(
                    np.asarray(src_feature[n], dtype=np.float32).reshape(C, HW)
                ),
                "ref": np.ascontiguousarray(
                    np.asarray(ref_feature[n], dtype=np.float32).reshape(C, HW)
                ),
                "mask": np.ascontiguousarray(mask_bf[n]),
                "wT": wT,
            }
        )
    return maps


def _install_ntff_hook():
    """The agent image's antenv lacks axon_hooks; recreate it so
    run_bass_kernel_spmd(trace=True) can capture NTFF profiles."""
    import sys
    import types

    if "antenv.axon_hooks" in sys.modules:
        return
    import antenv
    from trn_agent_boot.trn_boot import _ntff_profile_via_ctypes

    hook = _ntff_profile_via_ctypes("/opt/axon/libaxon_pjrt.so")
    mod = types.ModuleType("antenv.axon_hooks")
    mod._hook = hook
    mod.set_axon_ntff_profile_hook = lambda h: setattr(mod, "_hook", h)
    mod.get_axon_ntff_profile_hook = lambda: mod._hook
    sys.modules["antenv.axon_hooks"] = mod
    antenv.axon_hooks = mod


def run(src_mask, src_feature, ref_feature, conv_w, trace=False):
    """Run on 8 NeuronCores. Returns (output [N,2C,H,W], BassKernelResults)."""
    n_batch, c, h, w = src_feature.shape
    if trace:
        _install_ntff_hook()
    nc = _get_nc()
    maps = _in_maps(src_mask, src_feature, ref_feature, conv_w)
    res = bass_utils.run_bass_kernel_spmd(
        nc, maps, core_ids=list(range(NCORES)), trace=trace
    )
    out = np.stack([np.asarray(r["out"]) for r in res.results], axis=0)
    return out.reshape(n_batch, 2 * c, h, w).astype(np.float32), res


def kernel(src_mask, src_feature, ref_feature, conv_w):
    out, _ = run(src_mask, src_feature, ref_feature, conv_w)
    return out



# revision 76
# speedup vs baseline: 1.1418x; 1.0025x over previous
"""Trainium2 Bass kernel for ExampleGuidedAttention (N=8, C=256, H=W=64).

Data-parallel over batch N across 8 NeuronCores; each core computes one
batch element's full guided attention.

Algorithm notes (per core):
  q = conv_w @ src_pix                      [64, 4096]   (PE, bf16)
  S^T[j,i] = sum_o q[o,j] q[o,i]            (PE, bf16; S symmetric; two
             j-blocks packed in the 128x128 array via tile_position
             (row groups 0-63 / 64-127) since the contraction is only 64)
  F[j,i] = exp(S^T[j,i] - 64)               (ACT; global shift keeps fp32
             exp in range -- softmax ratio unchanged; diag scores are
             chi2(64) so they reach ~120).  The ACT accumulator
             (accum_out) yields Z partials for free: Z[j] = sum_i F[j,i]
             equals the softmax denominator because S is symmetric.
  O[c,i] = sum_j pixT[j,c] * F[j,i]         (PE, bf16, natural layout)
  out    = [ (1-m)*ref_att*invZ + m*ref ; src_att*invZ ]

Performance structure (~326us vs the 372us v2 kernel):
  - inputs land as 8KB-contiguous partition lines ([128, 2048] f32
    convert chunks on the gpsimd queue, ~235GB/s): 16KB lines silently
    corrupt; bf16 static loads (4KB lines) run at only ~70GB/s, so f32 +
    cast-DMA is the FAST input path.  One queue, priority order (src,
    mask, ref) -- the DMA engines fan out a single queue anyway.
  - exp runs as ONE fused [128,1024] ACTIVATE per scores pair over a
    2-bank PSUM pair tile ((N+352)/1.2ns); Z row-sums alternate between
    the ACT accumulator (unfused per-jb exps, odd pairs) and DVE
    reduces (even pairs) so NEITHER engine saturates.  Never recover a
    pair's Z as "mixed-accum minus one half": Z spans e^+-34 and the
    subtraction cancels catastrophically.
  - scores pairs ride inside the conv loop (2 per conv slice, ACT- and
    DMA-paced together), sc(1) inside apply(0), sc(2)+sc(3) inside
    apply(1), then lookahead-2; slices 0/3 are all-fused (ACT-lighter)
    with their reduces deferred to roomier DVE windows.
  - apply(0) splits into a src half (starts right after src lands; no
    ref dependency) and a ref half that hosts the tref transposes once
    ref has fully landed.  apply(7) is cb-major so each output row
    drains + stores under the next row's matmul stream.
  - PE warm-up uses RANDOM data: constant warm operands defeat the PE
    power estimator (HAM reads 8/8 but the whole run executes at
    ~1.2GHz, +60us).  warm_fill(32) bridges exactly to the first src
    chunk; too short re-throttles (+10us), too long delays conv.
  - invz/izb chain runs 2 slices early (zpart complete once exp(7)
    retires during apply(5)), so the one-shot [128,3584] finalize +
    output DMAs (7KB lines!) hide under apply(6)/apply(7).
  - everything downstream of PSUM is bf16 (output DRAM tensor too;
    host casts back to f32) -- halves DVE and output-DMA cost.
"""

import numpy as np

import concourse.bass as bass
import concourse.mybir as mybir
import concourse.tile as tile
from concourse import bacc, bass_utils
from concourse.bass import ts
from concourse.masks import make_identity

P = 128
C = 256          # feature channels
CQ = 64          # query channels
HW = 4096        # pixels per image
NB = HW // P     # 32 pixel blocks (contraction chunks)
SLICE = 512
NS = HW // SLICE  # 8 output column slices
NCORES = 8

F32 = mybir.dt.float32
BF16 = mybir.dt.bfloat16
EXP = mybir.ActivationFunctionType.Exp
COPY = mybir.ActivationFunctionType.Copy
AX_X = mybir.AxisListType.X


def _build_body(tc, src, ref, mask, wT, out, dbg=None):
    nc = tc.nc
    src_r = src.ap().rearrange("(ci p) j -> p ci j", p=P)   # [128, 2, 4096]
    ref_r = ref.ap().rearrange("(ci p) j -> p ci j", p=P)
    wT_r = wT.ap().rearrange("(ci p) o -> p ci o", p=P)     # [128, 2, 64]
    out_r = out.ap().rearrange("(cb p) j -> cb p j", p=P)   # [4, 128, 4096]

    with (
        tc.tile_pool(name="persist", bufs=1) as persist,
        tc.tile_pool(name="ps_s", bufs=2, space="PSUM") as ps_s,
        tc.tile_pool(name="ps_o", bufs=4, space="PSUM") as ps_o,
    ):
        # q duplicated into both partition halves so scores matmuls can be
        # row-packed: tile at rows 0-63 and rows 64-127 run concurrently.
        q2 = persist.tile([P, HW], BF16)
        pixT_src = persist.tile([P, NB, C], BF16)
        pixT_ref = persist.tile([P, NB, C], BF16)
        wT_sb = persist.tile([P, 2, CQ], BF16)
        # Z row-sums per scores-pair come from ONE DVE reduce over the fused
        # [128, 2, 512] exp tile (axis X keeps the two j-blocks separate).
        # NOTE: do NOT try to split this as "ACT-accum mixed sum minus one
        # DVE half" -- Z magnitudes span e^+-34 across rows, so Z_jb1 =
        # M - Z_jb0 cancels catastrophically whenever Z_jb0 >> Z_jb1.
        zpart = persist.tile([P, NB // 2, 2, NS], F32)
        z_all = persist.tile([P, NB // 2, 2], F32)
        invz = persist.tile([P, NB // 2, 2], F32)
        onem = persist.tile([P, HW], BF16)       # (1 - mask) broadcast
        # refb holds ref during the tref transposes, then is overwritten in
        # place with m*ref (the blend addend) by the mask-broadcast drains
        # -- no separate m_rep / tmpf tiles (SBUF).
        refb = persist.tile([P, 2, HW], BF16)
        izb = persist.tile([P, HW], BF16)        # 1/Z broadcast
        o_sb = persist.tile([P, 4, HW], BF16)
        # srcb lives in the persist pool: if it were freed before the f ring
        # allocates, the allocator recycles its SBUF region and exp(0)
        # inherits a false WAR on every srcb reader (conv + transposes),
        # which pushed the first ACTIVATE out to ~40us.
        srcb = persist.tile([P, 2, HW], BF16)
        exp_bias = persist.tile([P, 1], F32)
        ident = persist.tile([P, P], F32)
        identb = persist.tile([P, P], BF16)      # moving operand for PE transposes
        invz_T = persist.tile([NB, P], F32)
        ones_st = persist.tile([1, P], BF16)     # stationary ones row (K=1)
        warm_sb = persist.tile([P, SLICE], BF16) # zeroed filler operand
        mask_sb = persist.tile([1, HW], BF16)    # mask as a single row
        zrowb = persist.tile([1, HW], BF16)      # 1/Z as a single row
        act_warm = persist.tile([P, 1], F32)
        nc.vector.memset(exp_bias, -64.0)
        nc.vector.memset(ones_st, 1.0)
        make_identity(nc, ident)
        make_identity(nc, identb)
        # RANDOM warm data (max bit-toggle): constant-value warm operands
        # defeat the PE power estimator -- HAM reports 8/8 yet the whole
        # run executes at ~1.2GHz (+60us).  DVE random avoids the gpsimd
        # iota that would delay the input-DMA queue.
        nc.vector.random(warm_sb)
        # dummy exp: walrus inserts the ACT_TABLE_LOAD (~2.7us) before the
        # first Exp -- pay it here, under the input DMAs, not inside exp(0)
        nc.scalar.activation(out=act_warm, in_=exp_bias, func=EXP)


        if True:
            # PE warmup: back-to-back matmuls on constant nonzero data keep
            # the PE continuously busy until the first src chunk lands
            # (~13.5us) -- the HAM un-throttles to 8/8 after ~3.4us of
            # sustained activity, so conv/transposes/scores then run at
            # 2.4GHz instead of the cold 1.2GHz.
            def warm_fill(n):
                wp = ps_o.tile([P, SLICE], F32, name="warm", tag="pso")
                for _ in range(n):
                    nc.tensor.matmul(
                        wp, warm_sb[:, 0:P], warm_sb, start=True, stop=True
                    )

            # 44 matmuls span ~11us even if the first ~8 run cold (427ns at
            # 1.2GHz before the HAM lifts at ~3.4us) -- bridging the idle
            # window between warmup and the first src chunk (~14us).  A
            # shorter fill leaves a >2us idle there, and when the HAM's
            # free-running MID window catches it, the PE re-throttles to
            # 4/8 for ~34us right across conv/scores/apply(0) (+10us).
            warm_fill(32)

            # Input DMAs: ONE queue, priority order.  The DMA subsystem fans
            # one queue's descriptors across all 16 engines (~225GB/s
            # aggregate cap per core); multiple queues only make src, mask
            # and ref COMPETE so everything lands late together.  bf16
            # host-cast inputs halve the bytes vs the old f32 cast-DMAs:
            # src done ~11us, mask ~12, ref ~20.
            nc.sync.dma_start(out=wT_sb, in_=wT_r)
            JH = HW // 2
            for h in range(2):
                jh = slice(h * JH, (h + 1) * JH)
                for ci in range(2):
                    nc.gpsimd.dma_start(
                        out=srcb[:, ci, jh], in_=src_r[:, ci, jh]
                    )
            for h in range(2):
                jh = slice(h * JH, (h + 1) * JH)
                nc.gpsimd.dma_start(
                    out=mask_sb[:, jh],
                    in_=mask.ap()[jh].partition_broadcast(1),
                )
            for h in range(2):
                jh = slice(h * JH, (h + 1) * JH)
                for ci in range(2):
                    nc.gpsimd.dma_start(
                        out=refb[:, ci, jh], in_=ref_r[:, ci, jh]
                    )

            # pixT[j, c] = pix[c, j] via matmul(lhsT=pix-block, rhs=I):
            # out[j, c'] = sum_c pix[c, j] I[c, c'].  4 j-blocks share one
            # PSUM bank; one copy drains all 4.  The tiles live in the ps_o
            # ring (idle until apply(0)) so the conv/transpose chain never
            # shares the 2-deep pss ring with the ACT-paced scores pairs.
            # drain engine: DVE for the head (ACT is busy with exp(0)),
            # ACT for the tref groups inside apply(0) (DVE does Z there).
            def t_group(pix_in, pix_out, ci, g, drain):
                # head (drain=dve): ps_o ring -- free until apply(0), keeps
                # the conv/T chain off the scores ring.  inside apply(0)
                # (drain=act): ps_s ring -- ps_o's 4 slots are the apply
                # accumulators then.
                cs = slice(ci * P, (ci + 1) * P)
                if drain == "act":
                    psT = ps_s.tile([P, 4, P], F32, name="psT", tag="pss")
                else:
                    psT = ps_o.tile([P, 4, P], F32, name="psT", tag="pso")
                for q in range(4):
                    jb = g * 4 + q
                    nc.tensor.matmul(
                        psT[:, q, :], pix_in[:, ci, ts(jb, P)],
                        identb, start=True, stop=True,
                    )
                if drain == "act":
                    nc.scalar.activation(
                        out=pix_out[:, g * 4 : g * 4 + 4, cs], in_=psT,
                        func=COPY,
                    )
                else:
                    nc.vector.tensor_copy(
                        out=pix_out[:, g * 4 : g * 4 + 4, cs], in_=psT
                    )

        def scores_pair(s, f_sb, jp, fused=None):
            # Hybrid Z so neither ACT nor DVE saturates: even jp -> ONE
            # fused [128,1024] ACTIVATE (1147ns vs 2x720), Z row-sums later
            # via a DVE z_reduce; odd jp -> two per-jb ACTIVATEs whose ACT
            # accumulator yields clean per-jb Z for free (+294ns reads).
            # Per slice: ACT ~25us, DVE ~12us -- both under the ~34.5us
            # apply window.  (A fused ACTIVATE's accumulator would MIX the
            # two j-blocks' row sums, and un-mixing cancels catastrophically
            # since Z spans e^+-34 -- hence the per-jb unfused form.)
            sl = ts(s, SLICE)
            jb0, jb1 = 2 * jp, 2 * jp + 1
            pss = ps_s.tile([P, 2, SLICE], F32, name="pss", tag="pss")
            nc.tensor.matmul(
                pss[:, 0, :], q2[0:CQ, ts(jb0, P)], q2[0:CQ, sl],
                start=True, stop=True, tile_position=(0, 0),
            )
            nc.tensor.matmul(
                pss[:, 1, :], q2[CQ:P, ts(jb1, P)], q2[CQ:P, sl],
                start=True, stop=True, tile_position=(CQ, 0),
            )
            if fused is None:
                fused = jp % 2 == 0
            if fused:
                nc.scalar.activation(
                    out=f_sb[:, jb0 : jb0 + 2, :], in_=pss, func=EXP,
                    bias=exp_bias,
                )
            else:
                for h in range(2):
                    nc.scalar.activation(
                        out=f_sb[:, jb0 + h, :], in_=pss[:, h, :], func=EXP,
                        bias=exp_bias,
                        accum_out=zpart[:, jp, h, s : s + 1],
                    )

        def z_reduce(f_sb, s, jp):
            # Z row-sums for a FUSED pair (even jp) -- DVE, axis X keeps
            # the two j-blocks separate
            nc.vector.reduce_sum(
                out=zpart[:, jp, :, s : s + 1],
                in_=f_sb[:, 2 * jp : 2 * jp + 2, :],
                axis=AX_X,
            )

        def apply_mm(s, f_sb, mid_hook=None, next_scores=None, odd_hook=None,
                     zred=True):
            # jb-major; the NEXT slice's scores pairs interleave into the
            # stream one pair per two j-blocks (8 matmuls ~ 1.8us), which
            # matches the ACT exp drain rate -- the scores phase then
            # costs no standalone PE time and the pss pool never blocks.
            # jb-major keeps the f-ring WAR in lockstep: the pair for
            # j-blocks (2k, 2k+1) lands right after this slice's apply has
            # consumed those same blocks.  odd_hook(jb) lets extra work
            # (tref groups, a second scores set) slot into odd j-blocks.
            psos = [
                ps_o.tile([P, SLICE], F32, name=f"pso{cb}", tag="pso")
                for cb in range(4)
            ]
            for jb in range(NB):
                if jb == 12 and mid_hook is not None:
                    mid_hook()
                if next_scores is not None and jb % 2 == 0 and jb >= 2:
                    scores_pair(*next_scores, jb // 2 - 1)
                if zred and jb % 4 == 0:
                    # this slice's fused-pair Z reduces: pair jb//2 is read
                    # here just before exp(s+2) overwrites it at jb+2
                    z_reduce(f_sb, s, jb // 2)
                if odd_hook is not None and jb % 2 == 1:
                    odd_hook(jb)
                for cb in range(4):
                    pt = pixT_src if cb < 2 else pixT_ref
                    lhs = pt[:, jb, (cb % 2) * P : (cb % 2 + 1) * P]
                    nc.tensor.matmul(
                        psos[cb], lhs, f_sb[:, jb, :],
                        start=(jb == 0), stop=(jb == NB - 1),
                    )
            if next_scores is not None:
                scores_pair(*next_scores, NB // 2 - 1)
            return psos

        def copy_out(s, psos):
            # plain PSUM->SBUF copies; all mask/1/Z algebra happens in
            # finalize as  flow = ref + (1-m)*(ra*izb - ref)
            sl = ts(s, SLICE)
            for cb in range(4):
                nc.vector.tensor_copy(out=o_sb[:, cb, sl], in_=psos[cb])

        def izb_broadcast():
            # 1/Z row -> [128, HW] via ones-vector matmuls; ACT drains PSUM
            # (the DVE is busy with finalize at this point)
            for s2 in range(NS):
                sl2 = ts(s2, SLICE)
                psz = ps_s.tile([P, SLICE], F32, name="psz", tag="pss")
                nc.tensor.matmul(
                    psz, ones_st, zrowb[:, sl2], start=True, stop=True
                )
                nc.scalar.activation(out=izb[:, sl2], in_=psz, func=COPY)

        def finalize(lo, hi, dma_engines):
            """Normalize + blend + store for pixel columns [lo:hi).

            flow = (1-m)*ra*izb + m*ref  ==  ref + (1-m)*(ra*izb - ref),
            which needs only onem and RAW refb (no m*ref precompute)."""
            r = slice(lo, hi)
            for ci in range(2):
                o2 = o_sb[:, 2 + ci, r]
                nc.vector.tensor_mul(o2, o2, izb[:, r])
                nc.vector.tensor_sub(o2, o2, refb[:, ci, r])
                nc.vector.tensor_mul(o2, o2, onem[:, r])
                nc.vector.tensor_add(o2, o2, refb[:, ci, r])
                nc.vector.tensor_mul(o_sb[:, ci, r], o_sb[:, ci, r], izb[:, r])
            # out rows: [flow(=cb2,3), src_att(=cb0,1)]
            for k, cb in enumerate([2, 3, 0, 1]):
                eng = dma_engines[k % len(dma_engines)]
                eng.dma_start(out=out_r[k, :, r], in_=o_sb[:, cb, r])

        with tc.tile_pool(name="fbuf", bufs=2) as fbuf:
            # double-buffered F ring: exp(s+1) writes one buffer while
            # apply(s) streams the other.
            fbufs = [
                fbuf.tile([P, NB, SLICE], BF16, name="f_sb", tag="f")
                for _ in range(2)
            ]
            # conv + src transposes + scores(0), all paced by the src DMA:
            # conv slice s frees exactly the q columns that scores pairs
            # 2s/2s+1 need, so exp(0) STREAMS BEHIND THE SRC DMA and f(0)
            # is complete ~2us after src lands (vs +18us if scores waited
            # for the whole conv+transpose phase in the in-order PE queue).
            for s in range(NS):
                sl = ts(s, SLICE)
                psq = ps_o.tile([CQ, SLICE], F32, name="psq", tag="pso")
                for ci in range(2):
                    nc.tensor.matmul(
                        psq,
                        wT_sb[:, ci, :],
                        srcb[:, ci, sl],
                        start=(ci == 0),
                        stop=(ci == 1),
                    )
                nc.vector.tensor_copy(out=q2[0:CQ, sl], in_=psq)
                nc.vector.tensor_copy(out=q2[CQ:P, sl], in_=psq)
                t_group(srcb, pixT_src, 0, s, "dve")
                t_group(srcb, pixT_src, 1, s, "dve")
                # slice-0 exps ALL-FUSED: the hybrid's 25.4us of ACT per
                # slice would out-pace the src DMA here and delay apply(0);
                # fused is 18.4us.  Slice-0 Z reduces all happen on the DVE
                # inside apply(0)'s two halves.
                scores_pair(0, fbufs[0], 2 * s, fused=True)
                scores_pair(0, fbufs[0], 2 * s + 1, fused=True)

            # apply(0) is split into halves so it can START as soon as src
            # is done (~26us): the SRC half (cb0/1) needs only pixT_src +
            # f(0) and carries scores(1) + the slice-0 Z reduces; the REF
            # half starts ~19us later, by which time the whole ref tensor
            # has landed, so the tref transposes (odd j-blocks, one group
            # ahead of use) never stall on the ref DMA.
            sl0 = ts(0, SLICE)
            psosA = [
                ps_o.tile([P, SLICE], F32, name=f"ps0s{cb}", tag="pso")
                for cb in range(2)
            ]
            for jb in range(NB):
                if jb % 4 == 2:
                    scores_pair(1, fbufs[1], (jb - 2) // 4)
                if jb % 4 == 0:
                    z_reduce(fbufs[0], 0, jb // 2)
                for cb in range(2):
                    lhs = pixT_src[:, jb, (cb % 2) * P : (cb % 2 + 1) * P]
                    nc.tensor.matmul(
                        psosA[cb], lhs, fbufs[0][:, jb, :],
                        start=(jb == 0), stop=(jb == NB - 1),
                    )
            nc.vector.tensor_copy(out=o_sb[:, 0, sl0], in_=psosA[0])
            nc.vector.tensor_copy(out=o_sb[:, 1, sl0], in_=psosA[1])

            t_group(refb, pixT_ref, 0, 0, "act")
            t_group(refb, pixT_ref, 1, 0, "act")
            psosB = [
                ps_o.tile([P, SLICE], F32, name=f"ps0r{cb}", tag="pso")
                for cb in range(2)
            ]
            for jb in range(NB):
                if jb % 4 == 2:
                    scores_pair(1, fbufs[1], (jb - 2) // 4 + 8)
                if jb % 4 == 0:
                    z_reduce(fbufs[0], 0, jb // 2 + 1)
                if jb % 4 == 1 and jb // 4 + 1 < NB // 4:
                    t_group(refb, pixT_ref, 0, jb // 4 + 1, "act")
                elif jb % 4 == 3 and (jb - 3) // 4 + 1 < NB // 4:
                    t_group(refb, pixT_ref, 1, (jb - 3) // 4 + 1, "act")
                for cb in range(2):
                    lhs = pixT_ref[:, jb, cb * P : (cb + 1) * P]
                    nc.tensor.matmul(
                        psosB[cb], lhs, fbufs[0][:, jb, :],
                        start=(jb == 0), stop=(jb == NB - 1),
                    )
            nc.vector.tensor_copy(out=o_sb[:, 2, sl0], in_=psosB[0])
            nc.vector.tensor_copy(out=o_sb[:, 3, sl0], in_=psosB[1])
            # apply(1) carries TWO scores sets (sc2 even, sc3 odd) so the
            # steady lookahead-2 cadence resumes at apply(2) and scores(7)
            # still retires during apply(5) (the invz chain needs that).
            # sc(3) rides apply(1)'s odd j-blocks ALL-FUSED: apply(1)
            # already hosts exp(2)'s hybrid 25.4us; a second hybrid set
            # would put 50.8us of ACT into a ~36us window.  Slice-3's odd
            # Z reduces run on the DVE during apply(2) (even ones via
            # apply(3)'s standard zred hook).
            def sc3_hook(jb):
                if jb >= 3:
                    scores_pair(3, fbufs[1], (jb - 3) // 2, fused=True)

            psos = apply_mm(
                1, fbufs[1], next_scores=(2, fbufs[0]), odd_hook=sc3_hook
            )
            scores_pair(3, fbufs[1], NB // 2 - 1, fused=True)
            copy_out(1, psos)
            # mask column-broadcast (onem = 1-m) rides apply(3)'s odd
            # j-blocks -- it is only needed by the finalize at apply(6)
            def mask_hook(jb):
                s2 = jb // 2
                if jb % 2 == 1 and s2 < NS:
                    sl = ts(s2, SLICE)
                    psm = ps_s.tile([P, SLICE], F32, name="psm", tag="pss")
                    nc.tensor.matmul(
                        psm, ones_st, mask_sb[:, sl], start=True, stop=True
                    )
                    nc.scalar.activation(
                        out=onem[:, sl], in_=psm, func=COPY, scale=-1.0,
                        bias=1.0,
                    )

            def r3_hook(jb):
                if jb % 4 == 1:
                    z_reduce(fbufs[1], 3, (jb - 1) // 2 + 1)

            for s in range(2, NS - 2):
                if s == 2:
                    oh = r3_hook
                elif s == 3:
                    oh = mask_hook
                else:
                    oh = None
                psos = apply_mm(
                    s, fbufs[s % 2], next_scores=(s + 2, fbufs[s % 2]),
                    odd_hook=oh,
                )
                copy_out(s, psos)
            s7 = NS - 1
            f_cur = fbufs[(NS - 2) % 2]
            f_sb = fbufs[s7 % 2]
            # slices 6/7 are never overwritten, so their fused-pair Z
            # reduces run here (executing during apply(5)'s tail / the
            # apply(6) ramp); the unfused halves came from the ACT
            # accumulator at exp time.  Then the whole invz -> transpose ->
            # zrowb chain runs before apply(s6), so izb can broadcast
            # DURING apply(s6) and the DVE finalize of slices 0-6 + their
            # output DMAs hide under apply(s7).
            for jp in range(0, NB // 2, 2):
                z_reduce(f_cur, NS - 2, jp)
                z_reduce(f_sb, s7, jp)
            nc.vector.reduce_sum(out=z_all, in_=zpart, axis=AX_X)
            nc.vector.reciprocal(out=invz, in_=z_all)
            ps_t = ps_s.tile([NB, P], F32, name="ps_t", tag="pss")
            nc.tensor.transpose(
                ps_t, invz.rearrange("p a b -> p (a b)"), ident
            )
            nc.vector.tensor_copy(out=invz_T, in_=ps_t)
            # flatten [32 partitions, 128] -> one [1, 4096] row (SBUF->SBUF
            # DMA crosses partitions; f32 -> bf16 converts on the way)
            nc.gpsimd.dma_start(
                out=zrowb.rearrange("a (b q) -> a b q", q=P), in_=invz_T
            )
            psos = apply_mm(s7 - 1, f_cur, mid_hook=izb_broadcast, zred=False)
            copy_out(s7 - 1, psos)
            # per-slice finalize: each slice's 4 output DMAs fire as soon
            # as ITS DVE ops are done, so the ~4MB output streams out
            # under apply(7) instead of queueing behind one big finalize
            # one-shot finalize over slices 0-6: the [128, 3584] output
            # DMAs keep 7KB partition lines -- per-slice 512-col chunks
            # (1KB lines) drop the output DMA to ~70GB/s and cost +60us
            finalize(0, (NS - 1) * SLICE, [nc.sync, nc.scalar, nc.gpsimd])
            # slice 7 runs CB-MAJOR: each output row's 32-j-block psum
            # group completes after ~7us, so its normalize + blend + store
            # overlaps the next row's matmul stream -- only the last row's
            # short DVE/DMA chain remains after the final matmul.
            sl7 = ts(s7, SLICE)
            engs7 = [nc.sync, nc.scalar, nc.gpsimd, nc.sync]
            for k, cb in enumerate([2, 3, 0, 1]):
                pso = ps_o.tile([P, SLICE], F32, name=f"ps7{cb}", tag="pso")
                pt = pixT_src if cb < 2 else pixT_ref
                for jb in range(NB):
                    lhs = pt[:, jb, (cb % 2) * P : (cb % 2 + 1) * P]
                    nc.tensor.matmul(
                        pso, lhs, f_sb[:, jb, :],
                        start=(jb == 0), stop=(jb == NB - 1),
                    )
                o2 = o_sb[:, cb, sl7]
                nc.vector.tensor_mul(o2, pso, izb[:, sl7])
                if cb >= 2:
                    nc.vector.tensor_sub(o2, o2, refb[:, cb - 2, sl7])
                    nc.vector.tensor_mul(o2, o2, onem[:, sl7])
                    nc.vector.tensor_add(o2, o2, refb[:, cb - 2, sl7])
                engs7[k].dma_start(out=out_r[k, :, sl7], in_=o2)

            if dbg is not None:
                nc.sync.dma_start(out=dbg["q2"].ap(), in_=q2)
                nc.sync.dma_start(
                    out=dbg["zpart"].ap().rearrange("p (b s) -> p b s", s=NS),
                    in_=zpart,
                )
                nc.sync.dma_start(out=dbg["invz"].ap(), in_=invz)
                nc.sync.dma_start(out=dbg["izb"].ap(), in_=izb)
                nc.sync.dma_start(out=dbg["onem"].ap(), in_=onem)

                nc.sync.dma_start(
                    out=dbg["f7"].ap().rearrange("p (b i) -> p b i", b=NB),
                    in_=f_sb,
                )


def build():
    nc = bacc.Bacc(
        "TRN2",
        target_bir_lowering=False,
        debug=False,
        enable_asserts=False,
        num_devices=NCORES,
    )
    # src/ref stay f32 + gpsimd cast-DMAs: measured ~235GB/s with 8KB read
    # lines, vs ~70GB/s for bf16 static loads (4KB lines) -- the cast path
    # is the FAST one here.  mask is host-cast bf16 (tiny).
    src = nc.dram_tensor("src", (C, HW), F32, kind="ExternalInput")
    ref = nc.dram_tensor("ref", (C, HW), F32, kind="ExternalInput")
    mask = nc.dram_tensor("mask", (HW,), BF16, kind="ExternalInput")
    wT = nc.dram_tensor("wT", (C, CQ), BF16, kind="ExternalInput")
    out = nc.dram_tensor("out", (2 * C, HW), BF16, kind="ExternalOutput")
    with tile.TileContext(nc) as tc:
        _build_body(tc, src, ref, mask, wT, out)
    nc.compile()
    return nc


_CACHE = {}


def _get_nc():
    if "nc" not in _CACHE:
        _CACHE["nc"] = build()
    return _CACHE["nc"]


def _in_maps(src_mask, src_feature, ref_feature, conv_w):
    import ml_dtypes

    n_batch = src_feature.shape[0]
    wT = np.ascontiguousarray(
        np.asarray(conv_w, dtype=np.float32).T.astype(ml_dtypes.bfloat16)
    )
    mask_bf = np.asarray(src_mask, dtype=np.float32).reshape(
        n_batch, HW
    ).astype(ml_dtypes.bfloat16)
    maps = []
    for n in range(n_batch):
        maps.append(
            {
                "src": np.ascontiguousarray(
                    np.asarray(src_feature[n], dtype=np.float32).reshape(C, HW)
                ),
                "ref": np.ascontiguousarray(
                    np.asarray(ref_feature[n], dtype=np.float32).reshape(C, HW)
                ),
                "mask": np.ascontiguousarray(mask_bf[n]),
                "wT": wT,
            }
        )
    return maps


def _install_ntff_hook():
    """The agent image's antenv lacks axon_hooks; recreate it so
    run_bass_kernel_spmd(trace=True) can capture NTFF profiles."""
    import sys
    import types

    if "antenv.axon_hooks" in sys.modules:
        return
    import antenv
    from trn_agent_boot.trn_boot import _ntff_profile_via_ctypes

    hook = _ntff_profile_via_ctypes("/opt/axon/libaxon_pjrt.so")
    mod = types.ModuleType("antenv.axon_hooks")
    mod._hook = hook
    mod.set_axon_ntff_profile_hook = lambda h: setattr(mod, "_hook", h)
    mod.get_axon_ntff_profile_hook = lambda: mod._hook
    sys.modules["antenv.axon_hooks"] = mod
    antenv.axon_hooks = mod


def run(src_mask, src_feature, ref_feature, conv_w, trace=False):
    """Run on 8 NeuronCores. Returns (output [N,2C,H,W], BassKernelResults)."""
    n_batch, c, h, w = src_feature.shape
    if trace:
        _install_ntff_hook()
    nc = _get_nc()
    maps = _in_maps(src_mask, src_feature, ref_feature, conv_w)
    res = bass_utils.run_bass_kernel_spmd(
        nc, maps, core_ids=list(range(NCORES)), trace=trace
    )
    out = np.stack([np.asarray(r["out"]) for r in res.results], axis=0)
    return out.reshape(n_batch, 2 * c, h, w).astype(np.float32), res


def kernel(src_mask, src_feature, ref_feature, conv_w):
    out, _ = run(src_mask, src_feature, ref_feature, conv_w)
    return out

